# revision 44
# baseline (speedup 1.0000x reference)
"""Causal MHA (B=4, L=2048, D=1024, H=16) on 8 NeuronCores — fused pipeline.

Sharding: core c -> (batch b = c//2, head-group g = c%2), 8 heads/core.
wq/wk/wv column-parallel, wo row-parallel; host sums the two half-group
partials per batch and adds wo_b.

Single dataflow pipeline per core.  All four projection-class matmuls
(Q/K/V proj and the output projection C) run in fp8e4 DoubleRow perf mode
(0.5 PE cycles/row, contraction 2x128 per instr).  Precision is held at
~bf16 level with a 3-slot hi/lo decomposition: operand a = a_hi + a_lo
(both e4m3, host-split), product = ah*bh + al*bh + ah*bl (lo*lo dropped),
so each 8-ktile contraction costs 12 DR instrs = 6N cycles vs bf16's 8N.
Operands are pre-scaled by powers of 2 into e4m3's normal range; the
rescales fold into the exp scale (S arrives as 1024*S, exp applies
scale=1/1024), the vh ones-column (denominator carries the V scale so
ctx = num*rcp lands at 4*ctx, e4m3-ranged), and the C eviction (1/64).

Attention core: S = K^T@Q in f32r (1 cyc/row, 64-wide contraction), exp
on Act -> pt bf16, AV transposed (psum[q, 4, 65] += pt_chunk.T @ vh_kb)
in bf16, diag mask-mul on gpsimd (Pool is otherwise idle; DVE is loaded
with evictions).  The S/AV path cannot ride DoubleRow: fp8 quantization
of Q/K/P injects ~3-5% attention-weight noise, over the 2e-2 gate.

ctxT is produced as an fp8 hi/lo pair (DVE quantize + subtract off the
transpose psum) feeding the DR output projection.  Scheduling: flat item
stream with two filler queues — dq holds proj units with just-in-time
deadlines (per m-chunk: qt/kt chunk j lands right before head pair 2j;
V(n) before slice n's first AV drain, which lags 10 items so slice 0's V
can wait out its DMA), sq holds deferred C units popped when the Act
stream is ahead.  dq pops on deadline only: margin-popping would run
ahead of the serialized DMA stream and stall the in-order PE.  x tiles
are whole-slice single-DMA and prefetched a slice ahead; weights are one
DMA each (HWDGE costs ~665ns/DMA, so descriptor count matters).  Last-
slice C psums alternate into the freed S banks and their evictions
alternate Act/DVE to double the tail pipeline.  PSUM: 2x S sets (4
banks) + AV (1) + general (2) + transpose (1) = 8 banks.
TimelineSim: 212,788 ns (prior bf16 kernel: 245,461; rel err 4.6e-3).
"""

import numpy as np
import os as _os

import concourse.bacc as bacc
import concourse.bass as bass
import concourse.mybir as mybir
import concourse.tile as tile
from concourse.bass_utils import run_bass_kernel_spmd

F32 = mybir.dt.float32
F32R = mybir.dt.float32r
BF16 = mybir.dt.bfloat16
F8 = mybir.dt.float8e4
DR = mybir.MatmulPerfMode.DoubleRow

B, L, D, H, DK = 4, 2048, 1024, 16, 64
HD = 8              # heads per core
GW = 512            # head-group width
AUGW = HD * (DK + 1)  # 520
NCH = D // 128      # 8 contraction chunks
SL = 512            # token slice
NS = L // SL        # 4
NKB = L // 128      # 16

PE_NS = 1.0 / 2.4   # ns per PE cycle at full clock
ACT_NS = 1.0 / 1.2  # ns per Act cycle

SQ = 32.0 / (8.0 ** 0.5)   # scale folded into wq/wk before e4m3 split
SV = 32.0                  # scale folded into wv
SONE = 8.0                 # vh ones-column value -> ctx lands at 4*ctx
SO = 16.0                  # scale folded into wo
C_EVICT = 1.0 / 64.0       # (4*ctx)*(16*wo) -> /64
EXP_SCALE = 1.0 / 1024.0   # qt*kt = 1024*S_true


def _build_nc(dbg=False):
    nc = bacc.Bacc("TRN2", target_bir_lowering=False, debug=False, num_devices=8)

    # x/w tensors carry [hi; lo] e4m3 blocks stacked on the row (contraction)
    # axis; host does the split.
    xq = nc.dram_tensor("xq", [2 * D, L], F8, kind="ExternalInput").ap()
    xk = nc.dram_tensor("xk", [2 * D, L], F8, kind="ExternalInput").ap()
    xv = nc.dram_tensor("xv", [2 * D, L], F8, kind="ExternalInput").ap()
    wq = nc.dram_tensor("wq", [2 * D, GW], F8, kind="ExternalInput").ap()
    wk = nc.dram_tensor("wk", [2 * D, GW], F8, kind="ExternalInput").ap()
    wv = nc.dram_tensor("wv", [2 * D, AUGW], F8, kind="ExternalInput").ap()
    wo = nc.dram_tensor("wo", [2 * GW, D], F8, kind="ExternalInput").ap()
    bqk = nc.dram_tensor("bqk", [128, 8], F32, kind="ExternalInput").ap()
    vb = nc.dram_tensor("vb", [AUGW], F32, kind="ExternalInput").ap()
    mi = nc.dram_tensor("mi", [128, 256], BF16, kind="ExternalInput").ap()
    outp = nc.dram_tensor("outp", [L, D], BF16, kind="ExternalOutput").ap()
    if dbg:
        qt_dbg = nc.dram_tensor("qt_dbg", [128, 4 * L], F32, kind="ExternalOutput").ap()
        kt_dbg = nc.dram_tensor("kt_dbg", [128, 4 * L], F32, kind="ExternalOutput").ap()
        vh_dbg = nc.dram_tensor("vh_dbg", [128, NKB * 520], F32,
                                kind="ExternalOutput").ap()
        ctx_dbg = nc.dram_tensor("ctx_dbg", [128, 4 * L], F32,
                                 kind="ExternalOutput").ap()
        av_dbg = nc.dram_tensor("av_dbg", [128, HD * 4 * 65], F32,
                                kind="ExternalOutput").ap()
        ct_dbg = nc.dram_tensor("ct_dbg", [128, HD * 4 * DK], F32,
                                kind="ExternalOutput").ap()
        s_dbg = nc.dram_tensor("s_dbg", [128, HD * 1024], F32,
                               kind="ExternalOutput").ap()
        pt_dbg = nc.dram_tensor("pt_dbg", [128, HD * 1024], F32,
                                kind="ExternalOutput").ap()

    mask_eng_env = _os.environ.get("K_MASKDVE")

    with tile.TileContext(nc) as tc:
        with (
            tc.tile_pool(name="persist", bufs=1) as persist,
            tc.tile_pool(name="xin", bufs=4 if dbg else 6) as xinp,
            tc.tile_pool(name="pt", bufs=10) as ptp,
            tc.tile_pool(name="ctx", bufs=4) as ctxp,
            tc.tile_pool(name="ctxT8", bufs=4) as ctxT8p,
            tc.tile_pool(name="small", bufs=8) as smallp,
            tc.tile_pool(name="outs", bufs=4) as outsp,
            tc.tile_pool(name="psS", bufs=2, space="PSUM") as psS,
            tc.tile_pool(name="psAV", bufs=1, space="PSUM") as psAV,
            tc.tile_pool(name="psG", bufs=1, space="PSUM") as psG,
        ):
            # ---- persistent SBUF ----
            # weights: [128, s(hi/lo), ktile, cols]
            wq_s = persist.tile([128, 2, NCH, GW], F8, tag="wq")
            wk_s = persist.tile([128, 2, NCH, GW], F8, tag="wk")
            wv_s = persist.tile([128, 2, NCH, AUGW], F8, tag="wv")
            wo_s = persist.tile([128, 2, 4, D], F8, tag="wo")
            qt_s = persist.tile([128, 4, L], F32R, tag="qt")
            kt_s = persist.tile([128, 4, L], F32R, tag="kt")
            vh_s = persist.tile([128, NKB, 2, 260], BF16, tag="vh")
            bqk_s = persist.tile([128, 8], F32, tag="bqk")
            vb_s = persist.tile([128, AUGW], F32, tag="vb")
            mi_s = persist.tile([128, 256], BF16, tag="mi")

            def emit_w_dmas(which, split=False):
                if which == "bqk":
                    nc.sync.dma_start(bqk_s[:, :], bqk[:, :])
                elif which == "first":
                    nc.sync.dma_start(mi_s[:, :], mi[:, :])
                    vb_bcast = bass.AP(tensor=vb.tensor, offset=vb.offset,
                                       ap=[[0, 128], [1, AUGW]])
                    nc.sync.dma_start(vb_s[:, :], vb_bcast)
                elif which in ("q", "k", "v"):
                    w_s, w_d = {"q": (wq_s, wq), "k": (wk_s, wk),
                                "v": (wv_s, wv)}[which]
                    src = w_d.rearrange("(s c p) q -> p s c q", s=2, p=128)
                    if split:
                        for s in range(2):
                            nc.sync.dma_start(w_s[:, s, :, :], src[:, s, :, :])
                    else:
                        nc.sync.dma_start(w_s[:, :, :, :], src)
                else:
                    nc.sync.dma_start(
                        wo_s[:, :, :, :],
                        wo.rearrange("(s c p) q -> p s c q", s=2, p=128))

            # ---------- pacing counters (ns, at full clocks) ----------
            st = {"pe": 0.0, "act": 0.0}

            def mm(*args, **kw):
                out = args[0]
                st["pe"] += out.free_size() * PE_NS
                nc.tensor.matmul(*args, **kw)

            def mmdr(*args, **kw):
                out = args[0]
                st["pe"] += out.free_size() * 0.5 * PE_NS
                nc.tensor.matmul(*args, perf_mode=DR, **kw)

            # ---------- projection / output-projection units ----------
            def emit_x_dma(n, src, tag, split=False):
                # whole-slice tile [128, s(hi/lo), 8 ktiles, SL], one DMA
                # (two when split: hi first so class-A matmuls start earlier)
                t = xinp.tile([128, 2, NCH, SL], F8, tag="x", name=f"x{tag}{n}")
                s_ap = src.rearrange("(s c p) q -> p s c q", s=2, p=128)[
                    :, :, :, n * SL:(n + 1) * SL]
                if split:
                    for s in range(2):
                        nc.sync.dma_start(t[:, s, :, :], s_ap[:, s, :, :])
                else:
                    nc.sync.dma_start(t[:, :, :, :], s_ap)
                return t

            # 3-slot fp8 classes: (ws, xs) in A=(hi,hi), B=(lo,hi), C=(hi,lo)
            SLOT3 = ((0, 0), (1, 0), (0, 1))

            def emit_qk_unit(n, xt, w_s, dst, b_s, m, ci, psh):
                # ci: slot-class index 0..2 (emission granularity), or None
                cis = range(3) if ci is None else (ci,)
                if cis[0] == 0:
                    psh[m] = psG.tile([128, 512], F32, tag="g", bufs=2,
                                      name=f"qk{n}_{m}")
                ps = psh[m]
                for c3 in cis:
                    ws, xs = SLOT3[c3]
                    for cp in range(4):
                        cg = cp * 2
                        mmdr(ps[:, :],
                             w_s[:, ws, cg:cg + 2, m * 128:(m + 1) * 128],
                             xt[:, xs, cg:cg + 2, :],
                             start=(c3 == 0 and cp == 0),
                             stop=(c3 == 2 and cp == 3))
                if cis[-1] == 2:
                    del psh[m]
                    nc.vector.tensor_scalar_add(
                        dst[:, m, n * SL:(n + 1) * SL], ps[:, :],
                        b_s[:, m:m + 1])

            def emit_v_unit(n, xt, tt, hf, ci, psh):
                cis = range(3) if ci is None else (ci,)
                if cis[0] == 0:
                    psh[(tt, hf)] = psG.tile([128, 512], F32, tag="g", bufs=2,
                                             name=f"v{n}_{tt}_{hf}")
                ps = psh[(tt, hf)]
                for c3 in cis:
                    xs, ws = SLOT3[c3]  # lhsT is x here
                    for cp in range(4):
                        cg = cp * 2
                        mmdr(ps[:, 0:260],
                             xt[:, xs, cg:cg + 2, tt * 128:(tt + 1) * 128],
                             wv_s[:, ws, cg:cg + 2,
                                  hf * 260:(hf + 1) * 260],
                             start=(c3 == 0 and cp == 0),
                             stop=(c3 == 2 and cp == 3))
                if cis[-1] == 2:
                    del psh[(tt, hf)]
                    kb = n * 4 + tt
                    nc.vector.tensor_add(
                        vh_s[:, kb, hf, :],
                        ps[:, 0:260], vb_s[:, hf * 260:(hf + 1) * 260])

            def emit_c_unit(n, tt, n2, ctxT8_n):
                if n >= NS - 1 and (tt + n2) % 2 == 0:
                    # the S psum banks are free once the last exps retire;
                    # alternating pools doubles the tail eviction pipeline
                    ps2 = psS.tile([128, 2, 512], F32, tag="s",
                                   name=f"c{n}_{tt}_{n2}")
                    ps = ps2[:, 0, :]
                else:
                    ps = psG.tile([128, 512], F32, tag="g", bufs=2,
                                  name=f"c{n}_{tt}_{n2}")
                i = 0
                for s_ctx, s_wo in SLOT3:
                    for cp in range(2):
                        mmdr(ps[:, :],
                             ctxT8_n[:, s_ctx, cp * 2:cp * 2 + 2,
                                     tt * 128:(tt + 1) * 128],
                             wo_s[:, s_wo, cp * 2:cp * 2 + 2,
                                  n2 * 512:(n2 + 1) * 512],
                             start=(i == 0), stop=(i == 5))
                        i += 1
                ot = outsp.tile([128, 512], BF16, tag="ot", name=f"ot{n}_{tt}_{n2}")
                if n >= NS - 1 and (tt + n2) % 2 == 0:
                    # Act is done with exps by the time these pop; alternate
                    # Act/DVE so the psum-WAR recycle isn't one-queue-bound
                    nc.scalar.activation(ot[:, :], ps[:, :],
                                         func=mybir.ActivationFunctionType.Copy,
                                         scale=C_EVICT)
                else:
                    nc.vector.tensor_scalar_mul(ot[:, :], ps[:, :], C_EVICT)
                nc.sync.dma_start(
                    outp[(n * 4 + tt) * 128:(n * 4 + tt + 1) * 128,
                         n2 * 512:(n2 + 1) * 512], ot[:, :])

            def c_units(n, ctxT8_n):
                units = []
                for tt in range(4):
                    for n2 in range(2):
                        units.append(lambda n=n, tt=tt, n2=n2: emit_c_unit(
                            n, tt, n2, ctxT8_n))
                return units

            # ---------- prologue: DMAs + Q/K m0 only (act starts ASAP) ----
            # interleaved hi-first DMA order so class-A DR matmuls can start
            # after ~1MB of transfer
            emit_w_dmas("bqk")
            emit_w_dmas("first")
            wq_src = wq.rearrange("(s c p) q -> p s c q", s=2, p=128)
            wk_src = wk.rearrange("(s c p) q -> p s c q", s=2, p=128)
            xts = {}
            xts["q"] = xinp.tile([128, 2, NCH, SL], F8, tag="x", name="xq0")
            xts["k"] = xinp.tile([128, 2, NCH, SL], F8, tag="x", name="xk0")
            xq_src = xq.rearrange("(s c p) q -> p s c q", s=2, p=128)
            xk_src = xk.rearrange("(s c p) q -> p s c q", s=2, p=128)
            for s in range(2):
                nc.sync.dma_start(wq_s[:, s, :, :], wq_src[:, s, :, :])
                nc.sync.dma_start(xts["q"][:, s, :, :], xq_src[:, s, :, 0:SL])
            for s in range(2):
                nc.sync.dma_start(wk_s[:, s, :, :], wk_src[:, s, :, :])
                nc.sync.dma_start(xts["k"][:, s, :, :], xk_src[:, s, :, 0:SL])
            # full QK(0): the later m-chunks overlap the wk/xk DMA waits
            psh0 = {}
            for m in range(4):
                emit_qk_unit(0, xts["q"], wq_s, qt_s, bqk_s[:, 0:4], m, None, psh0)
            for m in range(4):
                emit_qk_unit(0, xts["k"], wk_s, kt_s, bqk_s[:, 4:8], m, None, psh0)
            emit_w_dmas("v")
            xts["v"] = emit_x_dma(0, xv, "v")

            # ---------- attention + pipeline ----------
            ctxT8_all = {}   # n -> [128, 2, 4, 512] fp8 hi/lo tile

            # Two filler queues paced against the Act exp stream:
            #  - dq: hard-deadline entries (due, thunk), FIFO in due order —
            #    projection units gated just-in-time per m-chunk / slice
            #  - sq: soft entries (deferred C units), popped on margin only
            dq = []
            sq = []
            margin = float(_os.environ.get("K_MARGIN", 2500))

            def pop_fillers(now=None, force_all=False):
                # dq pops strictly at deadline (margin pops would run ahead
                # of the DMA stream and stall the in-order PE); sq (C units)
                # absorbs the margin slack
                while dq and (force_all
                              or (now is not None and dq[0][0] <= now)):
                    dq.pop(0)[1]()
                while sq and (force_all or st["pe"] < st["act"] + margin):
                    sq.pop(0)()

            # prologue PE work ran concurrent with the x/w DMA stream; start
            # the pacing race fresh at the item stream
            st["act"] = st["pe"]

            def gidx(n, ii):
                return n * 1000 + ii

            def queue_qk_m(n1, m, xtd, psh, n_due, ii_due):
                # spread: 2 entries per item starting at (n_due, ii_due)
                ents = []
                for ci in range(3):
                    ents.append(lambda m=m, ci=ci: emit_qk_unit(
                        n1, xtd["q"], wq_s, qt_s, bqk_s[:, 0:4], m, ci, psh))
                for ci in range(3):
                    ents.append(lambda m=m, ci=ci: emit_qk_unit(
                        n1, xtd["k"], wk_s, kt_s, bqk_s[:, 4:8], m, ci, psh))
                for i, e in enumerate(ents):
                    dq.append((gidx(n_due, ii_due + i // 2), e))

            def queue_v(n1, xtd, psh, n_due, ii_due, spread=2):
                # AV consumes the diagonal key blocks (high tt) first and
                # heads 0-3 (hf=0) before 4-7
                ents = [lambda tt=tt, hf=hf: emit_v_unit(
                            n1, xtd["v"], tt, hf, None, psh)
                        for tt in range(4) for hf in range(2)]
                for i, e in enumerate(ents):
                    dq.append((gidx(n_due, ii_due + i // spread), e))

            # V(0) just-in-time within slice 0 (after its DMA lands); wo
            # rides the queue behind the slice-1 x prefetch
            # V(0) pops at items 5-8: after the wv/xv0 DMAs land (no in-order
            # PE stall) and before the first AV drain (at item 10)
            psh0b = {}
            queue_v(0, xts, psh0b, 0, 5)
            dq.append((gidx(0, 9), lambda: emit_w_dmas("rest")))

            for n in range(NS):
                if n + 1 < NS:
                    # eager whole-slice x prefetch + JIT-gated proj units;
                    # V(n+1)+m0(n+1) spread over the tail items of slice n
                    n1 = n + 1
                    nxts = {"q": emit_x_dma(n1, xq, "q"),
                            "k": emit_x_dma(n1, xk, "k"),
                            "v": emit_x_dma(n1, xv, "v")}
                    pshn = {}
                    ni = HD * 2 * (n + 1)    # items in slice n
                    npg = 2 * (n1 + 1)       # items per head in slice n1
                    queue_v(n1, nxts, pshn, n, ni - 7)
                    queue_qk_m(n1, 0, nxts, pshn, n, ni - 3)
                    for j in (1, 2, 3):
                        queue_qk_m(n1, j, nxts, pshn,
                                   n1, max(0, 2 * j * npg - 7))

                nkb = 4 * n + 4
                ngrp = nkb // 2
                ctxT8_n = ctxT8p.tile([128, 2, 4, 512], F8, tag="ctxT8",
                                      name=f"ctxT8_{n}")
                ctxT8_all[n] = ctxT8_n
                psT_cur = [None]

                def emit_s_exp(h, g, pts):
                    # Both banks of a group share the group's column base so a
                    # single 2-bank exp covers them (the extra computed scores
                    # land in q-chunks the AV stage never reads).
                    mc, po = h // 2, (h % 2) * 64
                    sps = psS.tile([128, 2, 512], F32, tag="s",
                                   name=f"s{n}_{h}_{g}")
                    pt = ptp.tile([128, 2, 512], BF16, tag="pt",
                                  name=f"pt{n}_{h}_{g}")
                    c0a = max(0, 2 * g * 128 - n * SL)
                    for i in range(2):
                        kb = 2 * g + i
                        # per-kb trim: queries before the key block are dead
                        # (AV skips them); exp still reads from c0a, the
                        # stale-psum cols it covers land in dead pt slots.
                        # f32r needs >=256 moving cols for 1 cyc/row.
                        c0i = min(max(0, kb * 128 - n * SL), SL - 256)
                        mm(sps[:, i, c0i:],
                           kt_s[po:po + 64, mc, kb * 128:(kb + 1) * 128],
                           qt_s[po:po + 64, mc, n * SL + c0i:(n + 1) * SL],
                           start=True, stop=True)
                    if dbg and n == 0 and g == 0:
                        sd = outsp.tile([128, 2, 512], F32, tag="sd", bufs=1,
                                        name=f"sd{n}_{h}_{g}")
                        nc.vector.tensor_copy(sd[:, :, :], sps[:, :, :])
                        nc.sync.dma_start(
                            s_dbg[:, h * 1024:(h + 1) * 1024], sd[:, :, :])
                    st["act"] += (2 * (512 - c0a)) * ACT_NS + 185.0
                    nc.scalar.activation(
                        pt[:, :, c0a:], sps[:, :, c0a:],
                        func=mybir.ActivationFunctionType.Exp,
                        scale=EXP_SCALE)
                    if dbg and n == 0 and g == 0:
                        pd = outsp.tile([128, 2, 512], F32, tag="pd", bufs=1,
                                        name=f"pd{n}_{h}_{g}")
                        nc.vector.tensor_copy(pd[:, :, :], pt[:, :, :])
                        nc.sync.dma_start(
                            pt_dbg[:, h * 1024:(h + 1) * 1024], pd[:, :, :])
                    pts[(h, g)] = pt

                def emit_av(h, g, avp, pts, first_grp, last_grp):
                    # PSUM start=True lazily zero-marks the WHOLE bank, so
                    # only the first emitted write into the bank may use it;
                    # later first-writes per region overwrite via the
                    # pending-zero flags.  Accumulation order over kb is free.
                    pt = pts.pop((h, g))
                    for i in range(2):
                        kb = 2 * g + i
                        if kb >= 4 * n:  # diagonal block: causal mask
                            col0 = max(0, kb * 128 - n * SL)
                            (nc.vector if mask_eng_env
                             else nc.gpsimd).tensor_mul(
                                pt[:, i, col0:col0 + 128],
                                pt[:, i, col0:col0 + 128], mi_s[:, 0:128])
                    started = [not (g == first_grp)]
                    for i in range(2):
                        kb = 2 * g + i
                        for qc in range(4):
                            if kb > 4 * n + qc:
                                continue
                            last = (g == last_grp) and (
                                kb == min(1, 4 * n + qc))
                            mm(avp[:, qc, 0:65],
                               pt[:, i, qc * 128:(qc + 1) * 128],
                               vh_s[:, kb, h // 4, (h % 4) * 65:(h % 4) * 65 + 65],
                               start=not started[0],
                               stop=last,
                               skip_group_check=True)
                            started[0] = True

                def emit_tail(h, avp):
                    # normalize: ctx_t[q, d] = 4 * av[q, d] / (av[q, 64]/8)
                    mc, po = h // 2, (h % 2) * 64
                    rcp = smallp.tile([128, 4, 1], F32, tag="rcp",
                                      name=f"rcp{n}_{h}")
                    nc.vector.reciprocal(rcp[:, :], avp[:, :, 64:65])
                    ctx_t = ctxp.tile([128, 4, DK], BF16, tag="ctx",
                                      name=f"ctx{n}_{h}")
                    if dbg and n == 0:
                        avd = outsp.tile([128, 4, 65], F32, tag="avd", bufs=1,
                                         name=f"avd{n}_{h}")
                        nc.vector.tensor_copy(avd[:, :, :], avp[:, :, :65])
                        nc.sync.dma_start(
                            av_dbg[:, h * 260:(h + 1) * 260], avd[:, :, :])
                    for qc in range(4):
                        nc.vector.tensor_scalar_mul(
                            ctx_t[:, qc, :], avp[:, qc, 0:DK], rcp[:, qc, :])
                    if dbg and n == 0:
                        ctd = outsp.tile([128, 4, DK], F32, tag="ctd", bufs=1,
                                         name=f"ctd{n}_{h}")
                        nc.vector.tensor_copy(ctd[:, :, :], ctx_t[:, :, :])
                        nc.sync.dma_start(
                            ct_dbg[:, h * 256:(h + 1) * 256], ctd[:, :, :])
                    # transpose to dims-major; 2 heads share one psum tile
                    if h % 2 == 0:
                        psT_cur[0] = psG.tile([128, 512], BF16, tag="t",
                                              name=f"t{n}_{mc}")
                    psT = psT_cur[0]
                    for qc in range(4):
                        st["pe"] += 128 * PE_NS
                        nc.tensor.transpose(
                            psT[po:po + 64, qc * 128:(qc + 1) * 128],
                            ctx_t[:, qc, :], mi_s[:, 128:256])
                    if h % 2 == 1:
                        # fp8 hi/lo split of the transposed ctx chunk
                        nc.vector.tensor_copy(ctxT8_n[:, 0, mc, :], psT[:, :])
                        nc.vector.tensor_sub(ctxT8_n[:, 1, mc, :], psT[:, :],
                                             ctxT8_n[:, 0, mc, :])

                # flat (head, group) pipeline: AV lags S/exp by one item so
                # head boundaries don't bunch the Act queue against psS WARs
                gorder = list(range(ngrp - 1, -1, -1))  # diag groups first
                items = [(h, g) for h in range(HD) for g in gorder]
                pts = {}
                avps = {}
                pending = []

                def drain_av():
                    ph, pg = pending.pop(0)
                    emit_av(ph, pg, avps[ph], pts, gorder[0], gorder[-1])
                    if pg == gorder[-1]:
                        emit_tail(ph, avps.pop(ph))

                for ii, (h, g) in enumerate(items):
                    if g == gorder[0]:
                        avps[h] = psAV.tile([128, 4, 128], F32, tag="av",
                                            name=f"av{n}_{h}")
                    emit_s_exp(h, g, pts)
                    pending.append((h, g))
                    if len(pending) > 9:
                        drain_av()
                    pop_fillers(now=gidx(n, ii))
                while pending:
                    drain_av()

                # C units for this slice become available now
                sq.extend(c_units(n, ctxT8_n))
                if dbg:
                    for c in range(4):
                        ct = outsp.tile([128, 512], F32, tag="dbg", bufs=1,
                                        name=f"cdb{n}_{c}")
                        h8 = outsp.tile([128, 512], F32, tag="dbg8", bufs=1,
                                        name=f"cdb8{n}_{c}")
                        nc.vector.tensor_copy(h8[:, :], ctxT8_n[:, 0, c, :])
                        nc.vector.tensor_add(ct[:, :], h8[:, :],
                                             ctxT8_n[:, 1, c, :])
                        nc.sync.dma_start(
                            ctx_dbg[:, (n * 4 + c) * 512:(n * 4 + c + 1) * 512],
                            ct[:, :])

            # ---------- drain remaining fillers (incl. all deferred C) ----
            pop_fillers(force_all=True)

            if dbg:
                nc.sync.dma_start(qt_dbg[:, :], qt_s[:, :, :].bitcast(F32))
                nc.sync.dma_start(kt_dbg[:, :], kt_s[:, :, :].bitcast(F32))
                for kb in range(NKB):
                    for hf in range(2):
                        vt = outsp.tile([128, 512], F32, tag="dbg", bufs=1,
                                        name=f"vdb{kb}_{hf}")
                        nc.vector.tensor_copy(vt[:, 0:260], vh_s[:, kb, hf, :])
                        nc.sync.dma_start(
                            vh_dbg[:, kb * 520 + hf * 260:
                                   kb * 520 + (hf + 1) * 260], vt[:, 0:260])

    nc.compile()
    return nc


_NC = None
LAST_RESULTS = None


def _hl(x, f8):
    h = x.astype(f8)
    l = (x - h.astype(np.float32)).astype(f8)
    return np.concatenate([h, l], axis=0)


def kernel(**inputs):
    global _NC, LAST_RESULTS
    import os
    import ml_dtypes
    if _NC is None:
        _NC = _build_nc()

    f8 = ml_dtypes.float8_e4m3
    bf = ml_dtypes.bfloat16
    f = lambda a: np.asarray(a, dtype=np.float32)
    q, k, v = f(inputs["q"]), f(inputs["k"]), f(inputs["v"])
    wq_w, wq_b = f(inputs["wq_w"]), f(inputs["wq_b"])
    wk_w, wk_b = f(inputs["wk_w"]), f(inputs["wk_b"])
    wv_w, wv_b = f(inputs["wv_w"]), f(inputs["wv_b"])
    wo_w, wo_b = f(inputs["wo_w"]), f(inputs["wo_b"])

    msk = np.ascontiguousarray(
        (np.arange(128)[None, :] >= np.arange(128)[:, None])).astype(bf)
    idn = np.eye(128).astype(bf)
    mi = np.ascontiguousarray(np.concatenate([msk, idn], axis=1))

    gmaps = []
    for g in range(2):
        sl = slice(g * GW, (g + 1) * GW)
        wq8 = _hl(np.ascontiguousarray(wq_w[sl].T * SQ), f8)
        wk8 = _hl(np.ascontiguousarray(wk_w[sl].T * SQ), f8)
        wvT = np.zeros((D, AUGW), np.float32)
        vbias = np.zeros((AUGW,), np.float32)
        for h in range(HD):
            wvT[:, h * 65:h * 65 + 64] = wv_w[g * GW + h * 64:
                                              g * GW + (h + 1) * 64].T * SV
            vbias[h * 65:h * 65 + 64] = wv_b[g * GW + h * 64:
                                             g * GW + (h + 1) * 64] * SV
            vbias[h * 65 + 64] = SONE
        wv8 = _hl(wvT, f8)
        wo8 = _hl(np.ascontiguousarray(wo_w[:, sl].T * SO), f8)
        bqT = np.ascontiguousarray((wq_b[sl] * SQ).reshape(4, 128).T)
        bkT = np.ascontiguousarray((wk_b[sl] * SQ).reshape(4, 128).T)
        gmaps.append(dict(wq=wq8, wk=wk8, wv=wv8, wo=wo8,
                          bqk=np.ascontiguousarray(
                              np.concatenate([bqT, bkT], axis=1)),
                          vb=vbias, mi=mi))

    bmaps = []
    for b in range(B):
        bmaps.append(dict(
            xq=_hl(np.ascontiguousarray(q[b].T), f8),
            xk=_hl(np.ascontiguousarray(k[b].T), f8),
            xv=_hl(np.ascontiguousarray(v[b].T), f8)))

    in_maps = [dict(**bmaps[c // 2], **gmaps[c % 2]) for c in range(8)]

    trace = bool(int(os.environ.get("KERNEL_TRACE", "0")))
    res = run_bass_kernel_spmd(_NC, in_maps, list(range(8)), trace=trace)
    LAST_RESULTS = res

    out = np.empty((B, L, D), np.float32)
    for b in range(B):
        out[b] = (np.asarray(res.results[2 * b]["outp"], np.float32)
                  + np.asarray(res.results[2 * b + 1]["outp"], np.float32)
                  + wo_b[None, :])
    return out


# revision 46
# speedup vs baseline: 1.0002x; 1.0002x over previous
"""Causal MHA (B=4, L=2048, D=1024, H=16) on 8 NeuronCores — fused pipeline.

Sharding: core c -> (batch b = c//2, head-group g = c%2), 8 heads/core.
wq/wk/wv column-parallel, wo row-parallel; host sums the two half-group
partials per batch and adds wo_b.

Single dataflow pipeline per core.  All four projection-class matmuls
(Q/K/V proj and the output projection C) run in fp8e4 DoubleRow perf mode
(0.5 PE cycles/row, contraction 2x128 per instr).  Precision is held at
~bf16 level with a 3-slot hi/lo decomposition: operand a = a_hi + a_lo
(both e4m3, host-split), product = ah*bh + al*bh + ah*bl (lo*lo dropped),
so each 8-ktile contraction costs 12 DR instrs = 6N cycles vs bf16's 8N.
Operands are pre-scaled by powers of 2 into e4m3's normal range; the
rescales fold into the exp scale (S arrives as 1024*S, exp applies
scale=1/1024), the vh ones-column (denominator carries the V scale so
ctx = num*rcp lands at 4*ctx, e4m3-ranged), and the C eviction (1/64).

Attention core: S = K^T@Q in f32r (1 cyc/row, 64-wide contraction), exp
on Act -> pt bf16, AV transposed (psum[q, 4, 65] += pt_chunk.T @ vh_kb)
in bf16, diag mask-mul on gpsimd (Pool is otherwise idle; DVE is loaded
with evictions).  The S/AV path cannot ride DoubleRow: fp8 quantization
of Q/K/P injects ~3-5% attention-weight noise, over the 2e-2 gate.

ctxT is produced as an fp8 hi/lo pair (DVE quantize + subtract off the
transpose psum) feeding the DR output projection.  Scheduling: flat item
stream with two filler queues — dq holds proj units with just-in-time
deadlines (per m-chunk: qt/kt chunk j lands right before head pair 2j;
V(n) before slice n's first AV drain, which lags 10 items so slice 0's V
can wait out its DMA), sq holds deferred C units popped when the Act
stream is ahead.  dq pops on deadline only: margin-popping would run
ahead of the serialized DMA stream and stall the in-order PE.  x tiles
are whole-slice single-DMA and prefetched a slice ahead; weights are one
DMA each (HWDGE costs ~665ns/DMA, so descriptor count matters).  Last-
slice C psums alternate into the freed S banks and their evictions
alternate Act/DVE to double the tail pipeline.  PSUM: 2x S sets (4
banks) + AV (1) + general (2) + transpose (1) = 8 banks.
TimelineSim: 212,788 ns (prior bf16 kernel: 245,461; rel err 4.6e-3).
"""

import numpy as np
import os as _os

import concourse.bacc as bacc
import concourse.bass as bass
import concourse.mybir as mybir
import concourse.tile as tile
from concourse.bass_utils import run_bass_kernel_spmd

F32 = mybir.dt.float32
F32R = mybir.dt.float32r
BF16 = mybir.dt.bfloat16
F8 = mybir.dt.float8e4
DR = mybir.MatmulPerfMode.DoubleRow

B, L, D, H, DK = 4, 2048, 1024, 16, 64
HD = 8              # heads per core
GW = 512            # head-group width
AUGW = HD * (DK + 1)  # 520
NCH = D // 128      # 8 contraction chunks
SL = 512            # token slice
NS = L // SL        # 4
NKB = L // 128      # 16

PE_NS = 1.0 / 2.4   # ns per PE cycle at full clock
ACT_NS = 1.0 / 1.2  # ns per Act cycle

SQ = 32.0 / (8.0 ** 0.5)   # scale folded into wq/wk before e4m3 split
SV = 32.0                  # scale folded into wv
SONE = 8.0                 # vh ones-column value -> ctx lands at 4*ctx
SO = 16.0                  # scale folded into wo
C_EVICT = 1.0 / 64.0       # (4*ctx)*(16*wo) -> /64
EXP_SCALE = 1.0 / 1024.0   # qt*kt = 1024*S_true


def _build_nc(dbg=False):
    nc = bacc.Bacc("TRN2", target_bir_lowering=False, debug=False, num_devices=8)

    # x/w tensors carry [hi; lo] e4m3 blocks stacked on the row (contraction)
    # axis; host does the split.
    xq = nc.dram_tensor("xq", [2 * D, L], F8, kind="ExternalInput").ap()
    xk = nc.dram_tensor("xk", [2 * D, L], F8, kind="ExternalInput").ap()
    xv = nc.dram_tensor("xv", [2 * D, L], F8, kind="ExternalInput").ap()
    wq = nc.dram_tensor("wq", [2 * D, GW], F8, kind="ExternalInput").ap()
    wk = nc.dram_tensor("wk", [2 * D, GW], F8, kind="ExternalInput").ap()
    wv = nc.dram_tensor("wv", [2 * D, AUGW], F8, kind="ExternalInput").ap()
    wo = nc.dram_tensor("wo", [2 * GW, D], F8, kind="ExternalInput").ap()
    bqk = nc.dram_tensor("bqk", [128, 8], F32, kind="ExternalInput").ap()
    vb = nc.dram_tensor("vb", [AUGW], F32, kind="ExternalInput").ap()
    mi = nc.dram_tensor("mi", [128, 256], BF16, kind="ExternalInput").ap()
    outp = nc.dram_tensor("outp", [L, D], BF16, kind="ExternalOutput").ap()
    if dbg:
        qt_dbg = nc.dram_tensor("qt_dbg", [128, 4 * L], F32, kind="ExternalOutput").ap()
        kt_dbg = nc.dram_tensor("kt_dbg", [128, 4 * L], F32, kind="ExternalOutput").ap()
        vh_dbg = nc.dram_tensor("vh_dbg", [128, NKB * 520], F32,
                                kind="ExternalOutput").ap()
        ctx_dbg = nc.dram_tensor("ctx_dbg", [128, 4 * L], F32,
                                 kind="ExternalOutput").ap()
        av_dbg = nc.dram_tensor("av_dbg", [128, HD * 4 * 65], F32,
                                kind="ExternalOutput").ap()
        ct_dbg = nc.dram_tensor("ct_dbg", [128, HD * 4 * DK], F32,
                                kind="ExternalOutput").ap()
        s_dbg = nc.dram_tensor("s_dbg", [128, HD * 1024], F32,
                               kind="ExternalOutput").ap()
        pt_dbg = nc.dram_tensor("pt_dbg", [128, HD * 1024], F32,
                                kind="ExternalOutput").ap()

    mask_eng_env = _os.environ.get("K_MASKDVE")

    with tile.TileContext(nc) as tc:
        with (
            tc.tile_pool(name="persist", bufs=1) as persist,
            tc.tile_pool(name="xin", bufs=4 if dbg else 6) as xinp,
            tc.tile_pool(name="pt", bufs=10) as ptp,
            tc.tile_pool(name="ctx", bufs=4) as ctxp,
            tc.tile_pool(name="ctxT8", bufs=4) as ctxT8p,
            tc.tile_pool(name="small", bufs=8) as smallp,
            tc.tile_pool(name="outs", bufs=4) as outsp,
            tc.tile_pool(name="psS", bufs=2, space="PSUM") as psS,
            tc.tile_pool(name="psAV", bufs=1, space="PSUM") as psAV,
            tc.tile_pool(name="psG", bufs=1, space="PSUM") as psG,
        ):
            # ---- persistent SBUF ----
            # weights: [128, s(hi/lo), ktile, cols]
            wq_s = persist.tile([128, 2, NCH, GW], F8, tag="wq")
            wk_s = persist.tile([128, 2, NCH, GW], F8, tag="wk")
            wv_s = persist.tile([128, 2, NCH, AUGW], F8, tag="wv")
            wo_s = persist.tile([128, 2, 4, D], F8, tag="wo")
            qt_s = persist.tile([128, 4, L], F32R, tag="qt")
            kt_s = persist.tile([128, 4, L], F32R, tag="kt")
            vh_s = persist.tile([128, NKB, 2, 260], BF16, tag="vh")
            bqk_s = persist.tile([128, 8], F32, tag="bqk")
            vb_s = persist.tile([128, AUGW], F32, tag="vb")
            mi_s = persist.tile([128, 256], BF16, tag="mi")

            def emit_w_dmas(which, split=False):
                if which == "bqk":
                    nc.sync.dma_start(bqk_s[:, :], bqk[:, :])
                elif which == "first":
                    nc.sync.dma_start(mi_s[:, :], mi[:, :])
                    vb_bcast = bass.AP(tensor=vb.tensor, offset=vb.offset,
                                       ap=[[0, 128], [1, AUGW]])
                    nc.sync.dma_start(vb_s[:, :], vb_bcast)
                elif which in ("q", "k", "v"):
                    w_s, w_d = {"q": (wq_s, wq), "k": (wk_s, wk),
                                "v": (wv_s, wv)}[which]
                    src = w_d.rearrange("(s c p) q -> p s c q", s=2, p=128)
                    if split:
                        for s in range(2):
                            nc.sync.dma_start(w_s[:, s, :, :], src[:, s, :, :])
                    else:
                        nc.sync.dma_start(w_s[:, :, :, :], src)
                else:
                    nc.sync.dma_start(
                        wo_s[:, :, :, :],
                        wo.rearrange("(s c p) q -> p s c q", s=2, p=128))

            # ---------- pacing counters (ns, at full clocks) ----------
            st = {"pe": 0.0, "act": 0.0}

            def mm(*args, **kw):
                out = args[0]
                st["pe"] += out.free_size() * PE_NS
                nc.tensor.matmul(*args, **kw)

            def mmdr(*args, **kw):
                out = args[0]
                st["pe"] += out.free_size() * 0.5 * PE_NS
                nc.tensor.matmul(*args, perf_mode=DR, **kw)

            # ---------- projection / output-projection units ----------
            def emit_x_dma(n, src, tag, split=False):
                # whole-slice tile [128, s(hi/lo), 8 ktiles, SL], one DMA
                # (two when split: hi first so class-A matmuls start earlier)
                t = xinp.tile([128, 2, NCH, SL], F8, tag="x", name=f"x{tag}{n}")
                s_ap = src.rearrange("(s c p) q -> p s c q", s=2, p=128)[
                    :, :, :, n * SL:(n + 1) * SL]
                if split:
                    for s in range(2):
                        nc.sync.dma_start(t[:, s, :, :], s_ap[:, s, :, :])
                else:
                    nc.sync.dma_start(t[:, :, :, :], s_ap)
                return t

            # 3-slot fp8 classes: (ws, xs) in A=(hi,hi), B=(lo,hi), C=(hi,lo)
            SLOT3 = ((0, 0), (1, 0), (0, 1))

            def emit_qk_unit(n, xt, w_s, dst, b_s, m, ci, psh):
                # ci: slot-class index 0..2 (emission granularity), or None
                cis = range(3) if ci is None else (ci,)
                if cis[0] == 0:
                    psh[m] = psG.tile([128, 512], F32, tag="g", bufs=2,
                                      name=f"qk{n}_{m}")
                ps = psh[m]
                for c3 in cis:
                    ws, xs = SLOT3[c3]
                    for cp in range(4):
                        cg = cp * 2
                        mmdr(ps[:, :],
                             w_s[:, ws, cg:cg + 2, m * 128:(m + 1) * 128],
                             xt[:, xs, cg:cg + 2, :],
                             start=(c3 == 0 and cp == 0),
                             stop=(c3 == 2 and cp == 3))
                if cis[-1] == 2:
                    del psh[m]
                    nc.vector.tensor_scalar_add(
                        dst[:, m, n * SL:(n + 1) * SL], ps[:, :],
                        b_s[:, m:m + 1])

            def emit_v_unit(n, xt, tt, hf, ci, psh):
                cis = range(3) if ci is None else (ci,)
                if cis[0] == 0:
                    psh[(tt, hf)] = psG.tile([128, 512], F32, tag="g", bufs=2,
                                             name=f"v{n}_{tt}_{hf}")
                ps = psh[(tt, hf)]
                for c3 in cis:
                    xs, ws = SLOT3[c3]  # lhsT is x here
                    for cp in range(4):
                        cg = cp * 2
                        mmdr(ps[:, 0:260],
                             xt[:, xs, cg:cg + 2, tt * 128:(tt + 1) * 128],
                             wv_s[:, ws, cg:cg + 2,
                                  hf * 260:(hf + 1) * 260],
                             start=(c3 == 0 and cp == 0),
                             stop=(c3 == 2 and cp == 3))
                if cis[-1] == 2:
                    del psh[(tt, hf)]
                    kb = n * 4 + tt
                    nc.vector.tensor_add(
                        vh_s[:, kb, hf, :],
                        ps[:, 0:260], vb_s[:, hf * 260:(hf + 1) * 260])

            def emit_c_unit(n, tt, n2, ctxT8_n):
                if n >= NS - 1 and (tt + n2) % 2 == 0:
                    # the S psum banks are free once the last exps retire;
                    # alternating pools doubles the tail eviction pipeline
                    ps2 = psS.tile([128, 2, 512], F32, tag="s",
                                   name=f"c{n}_{tt}_{n2}")
                    ps = ps2[:, 0, :]
                else:
                    ps = psG.tile([128, 512], F32, tag="g", bufs=2,
                                  name=f"c{n}_{tt}_{n2}")
                i = 0
                for s_ctx, s_wo in SLOT3:
                    for cp in range(2):
                        mmdr(ps[:, :],
                             ctxT8_n[:, s_ctx, cp * 2:cp * 2 + 2,
                                     tt * 128:(tt + 1) * 128],
                             wo_s[:, s_wo, cp * 2:cp * 2 + 2,
                                  n2 * 512:(n2 + 1) * 512],
                             start=(i == 0), stop=(i == 5))
                        i += 1
                ot = outsp.tile([128, 512], BF16, tag="ot", name=f"ot{n}_{tt}_{n2}")
                if n >= NS - 1:
                    # Act is done with exps by the time these pop; DVE still
                    # has the last ctxT8 hi/lo work queued
                    nc.scalar.activation(ot[:, :], ps[:, :],
                                         func=mybir.ActivationFunctionType.Copy,
                                         scale=C_EVICT)
                else:
                    nc.vector.tensor_scalar_mul(ot[:, :], ps[:, :], C_EVICT)
                nc.sync.dma_start(
                    outp[(n * 4 + tt) * 128:(n * 4 + tt + 1) * 128,
                         n2 * 512:(n2 + 1) * 512], ot[:, :])

            def c_units(n, ctxT8_n):
                units = []
                for tt in range(4):
                    for n2 in range(2):
                        units.append(lambda n=n, tt=tt, n2=n2: emit_c_unit(
                            n, tt, n2, ctxT8_n))
                return units

            # ---------- prologue: DMAs + Q/K m0 only (act starts ASAP) ----
            # interleaved hi-first DMA order so class-A DR matmuls can start
            # after ~1MB of transfer
            emit_w_dmas("bqk")
            emit_w_dmas("first")
            wq_src = wq.rearrange("(s c p) q -> p s c q", s=2, p=128)
            wk_src = wk.rearrange("(s c p) q -> p s c q", s=2, p=128)
            xts = {}
            xts["q"] = xinp.tile([128, 2, NCH, SL], F8, tag="x", name="xq0")
            xts["k"] = xinp.tile([128, 2, NCH, SL], F8, tag="x", name="xk0")
            xq_src = xq.rearrange("(s c p) q -> p s c q", s=2, p=128)
            xk_src = xk.rearrange("(s c p) q -> p s c q", s=2, p=128)
            for s in range(2):
                nc.sync.dma_start(wq_s[:, s, :, :], wq_src[:, s, :, :])
                nc.sync.dma_start(xts["q"][:, s, :, :], xq_src[:, s, :, 0:SL])
            for s in range(2):
                nc.sync.dma_start(wk_s[:, s, :, :], wk_src[:, s, :, :])
                nc.sync.dma_start(xts["k"][:, s, :, :], xk_src[:, s, :, 0:SL])
            # full QK(0): the later m-chunks overlap the wk/xk DMA waits
            psh0 = {}
            for m in range(4):
                emit_qk_unit(0, xts["q"], wq_s, qt_s, bqk_s[:, 0:4], m, None, psh0)
            for m in range(4):
                emit_qk_unit(0, xts["k"], wk_s, kt_s, bqk_s[:, 4:8], m, None, psh0)
            emit_w_dmas("v")
            xts["v"] = emit_x_dma(0, xv, "v")

            # ---------- attention + pipeline ----------
            ctxT8_all = {}   # n -> [128, 2, 4, 512] fp8 hi/lo tile

            # Two filler queues paced against the Act exp stream:
            #  - dq: hard-deadline entries (due, thunk), FIFO in due order —
            #    projection units gated just-in-time per m-chunk / slice
            #  - sq: soft entries (deferred C units), popped on margin only
            dq = []
            sq = []
            margin = float(_os.environ.get("K_MARGIN", 2500))

            def pop_fillers(now=None, force_all=False):
                # dq pops strictly at deadline (margin pops would run ahead
                # of the DMA stream and stall the in-order PE); sq (C units)
                # absorbs the margin slack
                while dq and (force_all
                              or (now is not None and dq[0][0] <= now)):
                    dq.pop(0)[1]()
                while sq and (force_all or st["pe"] < st["act"] + margin):
                    sq.pop(0)()

            # prologue PE work ran concurrent with the x/w DMA stream; start
            # the pacing race fresh at the item stream
            st["act"] = st["pe"]

            def gidx(n, ii):
                return n * 1000 + ii

            def queue_qk_m(n1, m, xtd, psh, n_due, ii_due):
                # spread: 2 entries per item starting at (n_due, ii_due)
                ents = []
                for ci in range(3):
                    ents.append(lambda m=m, ci=ci: emit_qk_unit(
                        n1, xtd["q"], wq_s, qt_s, bqk_s[:, 0:4], m, ci, psh))
                for ci in range(3):
                    ents.append(lambda m=m, ci=ci: emit_qk_unit(
                        n1, xtd["k"], wk_s, kt_s, bqk_s[:, 4:8], m, ci, psh))
                for i, e in enumerate(ents):
                    dq.append((gidx(n_due, ii_due + i // 2), e))

            def queue_v(n1, xtd, psh, n_due, ii_due, spread=2):
                # AV consumes the diagonal key blocks (high tt) first and
                # heads 0-3 (hf=0) before 4-7
                ents = [lambda tt=tt, hf=hf: emit_v_unit(
                            n1, xtd["v"], tt, hf, None, psh)
                        for tt in range(4) for hf in range(2)]
                for i, e in enumerate(ents):
                    dq.append((gidx(n_due, ii_due + i // spread), e))

            # V(0) just-in-time within slice 0 (after its DMA lands); wo
            # rides the queue behind the slice-1 x prefetch
            # V(0) pops at items 5-8: after the wv/xv0 DMAs land (no in-order
            # PE stall) and before the first AV drain (at item 10)
            psh0b = {}
            queue_v(0, xts, psh0b, 0, 5)
            dq.append((gidx(0, 9), lambda: emit_w_dmas("rest")))

            for n in range(NS):
                if n + 1 < NS:
                    # eager whole-slice x prefetch + JIT-gated proj units;
                    # V(n+1)+m0(n+1) spread over the tail items of slice n
                    n1 = n + 1
                    nxts = {"q": emit_x_dma(n1, xq, "q"),
                            "k": emit_x_dma(n1, xk, "k"),
                            "v": emit_x_dma(n1, xv, "v")}
                    pshn = {}
                    ni = HD * 2 * (n + 1)    # items in slice n
                    npg = 2 * (n1 + 1)       # items per head in slice n1
                    queue_v(n1, nxts, pshn, n, ni - 7)
                    queue_qk_m(n1, 0, nxts, pshn, n, ni - 3)
                    for j in (1, 2, 3):
                        queue_qk_m(n1, j, nxts, pshn,
                                   n1, max(0, 2 * j * npg - 7))

                nkb = 4 * n + 4
                ngrp = nkb // 2
                ctxT8_n = ctxT8p.tile([128, 2, 4, 512], F8, tag="ctxT8",
                                      name=f"ctxT8_{n}")
                ctxT8_all[n] = ctxT8_n
                psT_cur = [None]

                def emit_s_exp(h, g, pts):
                    # Both banks of a group share the group's column base so a
                    # single 2-bank exp covers them (the extra computed scores
                    # land in q-chunks the AV stage never reads).
                    mc, po = h // 2, (h % 2) * 64
                    sps = psS.tile([128, 2, 512], F32, tag="s",
                                   name=f"s{n}_{h}_{g}")
                    pt = ptp.tile([128, 2, 512], BF16, tag="pt",
                                  name=f"pt{n}_{h}_{g}")
                    c0a = max(0, 2 * g * 128 - n * SL)
                    for i in range(2):
                        kb = 2 * g + i
                        # per-kb trim: queries before the key block are dead
                        # (AV skips them); exp still reads from c0a, the
                        # stale-psum cols it covers land in dead pt slots.
                        # f32r needs >=256 moving cols for 1 cyc/row.
                        c0i = min(max(0, kb * 128 - n * SL), SL - 256)
                        mm(sps[:, i, c0i:],
                           kt_s[po:po + 64, mc, kb * 128:(kb + 1) * 128],
                           qt_s[po:po + 64, mc, n * SL + c0i:(n + 1) * SL],
                           start=True, stop=True)
                    if dbg and n == 0 and g == 0:
                        sd = outsp.tile([128, 2, 512], F32, tag="sd", bufs=1,
                                        name=f"sd{n}_{h}_{g}")
                        nc.vector.tensor_copy(sd[:, :, :], sps[:, :, :])
                        nc.sync.dma_start(
                            s_dbg[:, h * 1024:(h + 1) * 1024], sd[:, :, :])
                    st["act"] += (2 * (512 - c0a)) * ACT_NS + 185.0
                    nc.scalar.activation(
                        pt[:, :, c0a:], sps[:, :, c0a:],
                        func=mybir.ActivationFunctionType.Exp,
                        scale=EXP_SCALE)
                    if dbg and n == 0 and g == 0:
                        pd = outsp.tile([128, 2, 512], F32, tag="pd", bufs=1,
                                        name=f"pd{n}_{h}_{g}")
                        nc.vector.tensor_copy(pd[:, :, :], pt[:, :, :])
                        nc.sync.dma_start(
                            pt_dbg[:, h * 1024:(h + 1) * 1024], pd[:, :, :])
                    pts[(h, g)] = pt

                def emit_av(h, g, avp, pts, first_grp, last_grp):
                    # PSUM start=True lazily zero-marks the WHOLE bank, so
                    # only the first emitted write into the bank may use it;
                    # later first-writes per region overwrite via the
                    # pending-zero flags.  Accumulation order over kb is free.
                    pt = pts.pop((h, g))
                    for i in range(2):
                        kb = 2 * g + i
                        if kb >= 4 * n:  # diagonal block: causal mask
                            col0 = max(0, kb * 128 - n * SL)
                            (nc.vector if mask_eng_env
                             else nc.gpsimd).tensor_mul(
                                pt[:, i, col0:col0 + 128],
                                pt[:, i, col0:col0 + 128], mi_s[:, 0:128])
                    started = [not (g == first_grp)]
                    for i in range(2):
                        kb = 2 * g + i
                        for qc in range(4):
                            if kb > 4 * n + qc:
                                continue
                            last = (g == last_grp) and (
                                kb == min(1, 4 * n + qc))
                            mm(avp[:, qc, 0:65],
                               pt[:, i, qc * 128:(qc + 1) * 128],
                               vh_s[:, kb, h // 4, (h % 4) * 65:(h % 4) * 65 + 65],
                               start=not started[0],
                               stop=last,
                               skip_group_check=True)
                            started[0] = True

                def emit_tail(h, avp):
                    # normalize: ctx_t[q, d] = 4 * av[q, d] / (av[q, 64]/8)
                    mc, po = h // 2, (h % 2) * 64
                    rcp = smallp.tile([128, 4, 1], F32, tag="rcp",
                                      name=f"rcp{n}_{h}")
                    nc.vector.reciprocal(rcp[:, :], avp[:, :, 64:65])
                    ctx_t = ctxp.tile([128, 4, DK], BF16, tag="ctx",
                                      name=f"ctx{n}_{h}")
                    if dbg and n == 0:
                        avd = outsp.tile([128, 4, 65], F32, tag="avd", bufs=1,
                                         name=f"avd{n}_{h}")
                        nc.vector.tensor_copy(avd[:, :, :], avp[:, :, :65])
                        nc.sync.dma_start(
                            av_dbg[:, h * 260:(h + 1) * 260], avd[:, :, :])
                    for qc in range(4):
                        nc.vector.tensor_scalar_mul(
                            ctx_t[:, qc, :], avp[:, qc, 0:DK], rcp[:, qc, :])
                    if dbg and n == 0:
                        ctd = outsp.tile([128, 4, DK], F32, tag="ctd", bufs=1,
                                         name=f"ctd{n}_{h}")
                        nc.vector.tensor_copy(ctd[:, :, :], ctx_t[:, :, :])
                        nc.sync.dma_start(
                            ct_dbg[:, h * 256:(h + 1) * 256], ctd[:, :, :])
                    # transpose to dims-major; 2 heads share one psum tile
                    if h % 2 == 0:
                        psT_cur[0] = psG.tile([128, 512], BF16, tag="t",
                                              name=f"t{n}_{mc}")
                    psT = psT_cur[0]
                    for qc in range(4):
                        st["pe"] += 128 * PE_NS
                        nc.tensor.transpose(
                            psT[po:po + 64, qc * 128:(qc + 1) * 128],
                            ctx_t[:, qc, :], mi_s[:, 128:256])
                    if h % 2 == 1:
                        # fp8 hi/lo split of the transposed ctx chunk
                        nc.vector.tensor_copy(ctxT8_n[:, 0, mc, :], psT[:, :])
                        nc.vector.tensor_sub(ctxT8_n[:, 1, mc, :], psT[:, :],
                                             ctxT8_n[:, 0, mc, :])

                # flat (head, group) pipeline: AV lags S/exp by one item so
                # head boundaries don't bunch the Act queue against psS WARs
                gorder = list(range(ngrp - 1, -1, -1))  # diag groups first
                items = [(h, g) for h in range(HD) for g in gorder]
                pts = {}
                avps = {}
                pending = []

                def drain_av():
                    ph, pg = pending.pop(0)
                    emit_av(ph, pg, avps[ph], pts, gorder[0], gorder[-1])
                    if pg == gorder[-1]:
                        emit_tail(ph, avps.pop(ph))

                for ii, (h, g) in enumerate(items):
                    if g == gorder[0]:
                        avps[h] = psAV.tile([128, 4, 128], F32, tag="av",
                                            name=f"av{n}_{h}")
                    emit_s_exp(h, g, pts)
                    pending.append((h, g))
                    # deep lag lets slice-0's V wait out its DMA before the
                    # first AV; the last slice drains shallow so its tails
                    # (and so the C(3) units) finish earlier
                    if len(pending) > (9 if n < NS - 1 else 4):
                        drain_av()
                    pop_fillers(now=gidx(n, ii))
                while pending:
                    drain_av()

                # C units for this slice become available now
                sq.extend(c_units(n, ctxT8_n))
                if dbg:
                    for c in range(4):
                        ct = outsp.tile([128, 512], F32, tag="dbg", bufs=1,
                                        name=f"cdb{n}_{c}")
                        h8 = outsp.tile([128, 512], F32, tag="dbg8", bufs=1,
                                        name=f"cdb8{n}_{c}")
                        nc.vector.tensor_copy(h8[:, :], ctxT8_n[:, 0, c, :])
                        nc.vector.tensor_add(ct[:, :], h8[:, :],
                                             ctxT8_n[:, 1, c, :])
                        nc.sync.dma_start(
                            ctx_dbg[:, (n * 4 + c) * 512:(n * 4 + c + 1) * 512],
                            ct[:, :])

            # ---------- drain remaining fillers (incl. all deferred C) ----
            pop_fillers(force_all=True)

            if dbg:
                nc.sync.dma_start(qt_dbg[:, :], qt_s[:, :, :].bitcast(F32))
                nc.sync.dma_start(kt_dbg[:, :], kt_s[:, :, :].bitcast(F32))
                for kb in range(NKB):
                    for hf in range(2):
                        vt = outsp.tile([128, 512], F32, tag="dbg", bufs=1,
                                        name=f"vdb{kb}_{hf}")
                        nc.vector.tensor_copy(vt[:, 0:260], vh_s[:, kb, hf, :])
                        nc.sync.dma_start(
                            vh_dbg[:, kb * 520 + hf * 260:
                                   kb * 520 + (hf + 1) * 260], vt[:, 0:260])

    nc.compile()
    return nc


_NC = None
LAST_RESULTS = None


def _hl(x, f8):
    h = x.astype(f8)
    l = (x - h.astype(np.float32)).astype(f8)
    return np.concatenate([h, l], axis=0)


def kernel(**inputs):
    global _NC, LAST_RESULTS
    import os
    import ml_dtypes
    if _NC is None:
        _NC = _build_nc()

    f8 = ml_dtypes.float8_e4m3
    bf = ml_dtypes.bfloat16
    f = lambda a: np.asarray(a, dtype=np.float32)
    q, k, v = f(inputs["q"]), f(inputs["k"]), f(inputs["v"])
    wq_w, wq_b = f(inputs["wq_w"]), f(inputs["wq_b"])
    wk_w, wk_b = f(inputs["wk_w"]), f(inputs["wk_b"])
    wv_w, wv_b = f(inputs["wv_w"]), f(inputs["wv_b"])
    wo_w, wo_b = f(inputs["wo_w"]), f(inputs["wo_b"])

    msk = np.ascontiguousarray(
        (np.arange(128)[None, :] >= np.arange(128)[:, None])).astype(bf)
    idn = np.eye(128).astype(bf)
    mi = np.ascontiguousarray(np.concatenate([msk, idn], axis=1))

    gmaps = []
    for g in range(2):
        sl = slice(g * GW, (g + 1) * GW)
        wq8 = _hl(np.ascontiguousarray(wq_w[sl].T * SQ), f8)
        wk8 = _hl(np.ascontiguousarray(wk_w[sl].T * SQ), f8)
        wvT = np.zeros((D, AUGW), np.float32)
        vbias = np.zeros((AUGW,), np.float32)
        for h in range(HD):
            wvT[:, h * 65:h * 65 + 64] = wv_w[g * GW + h * 64:
                                              g * GW + (h + 1) * 64].T * SV
            vbias[h * 65:h * 65 + 64] = wv_b[g * GW + h * 64:
                                             g * GW + (h + 1) * 64] * SV
            vbias[h * 65 + 64] = SONE
        wv8 = _hl(wvT, f8)
        wo8 = _hl(np.ascontiguousarray(wo_w[:, sl].T * SO), f8)
        bqT = np.ascontiguousarray((wq_b[sl] * SQ).reshape(4, 128).T)
        bkT = np.ascontiguousarray((wk_b[sl] * SQ).reshape(4, 128).T)
        gmaps.append(dict(wq=wq8, wk=wk8, wv=wv8, wo=wo8,
                          bqk=np.ascontiguousarray(
                              np.concatenate([bqT, bkT], axis=1)),
                          vb=vbias, mi=mi))

    bmaps = []
    for b in range(B):
        bmaps.append(dict(
            xq=_hl(np.ascontiguousarray(q[b].T), f8),
            xk=_hl(np.ascontiguousarray(k[b].T), f8),
            xv=_hl(np.ascontiguousarray(v[b].T), f8)))

    in_maps = [dict(**bmaps[c // 2], **gmaps[c % 2]) for c in range(8)]

    trace = bool(int(os.environ.get("KERNEL_TRACE", "0")))
    res = run_bass_kernel_spmd(_NC, in_maps, list(range(8)), trace=trace)
    LAST_RESULTS = res

    out = np.empty((B, L, D), np.float32)
    for b in range(B):
        out[b] = (np.asarray(res.results[2 * b]["outp"], np.float32)
                  + np.asarray(res.results[2 * b + 1]["outp"], np.float32)
                  + wo_b[None, :])
    return out


# revision 51
# speedup vs baseline: 1.0046x; 1.0044x over previous
"""Causal MHA (B=4, L=2048, D=1024, H=16) on 8 NeuronCores — fused pipeline.

Sharding: core c -> (batch b = c//2, head-group g = c%2), 8 heads/core.
wq/wk/wv column-parallel, wo row-parallel; host sums the two half-group
partials per batch and adds wo_b.

Single dataflow pipeline per core.  All four projection-class matmuls
(Q/K/V proj and the output projection C) run in fp8e4 DoubleRow perf mode
(0.5 PE cycles/row, contraction 2x128 per instr).  Precision is held at
~bf16 level with a 3-slot hi/lo decomposition: operand a = a_hi + a_lo
(both e4m3, host-split), product = ah*bh + al*bh + ah*bl (lo*lo dropped),
so each 8-ktile contraction costs 12 DR instrs = 6N cycles vs bf16's 8N.
Operands are pre-scaled by powers of 2 into e4m3's normal range; the
rescales fold into the exp scale (S arrives as 1024*S, exp applies
scale=1/1024), the vh ones-column (denominator carries the V scale so
ctx = num*rcp lands at 4*ctx, e4m3-ranged), and the C eviction (1/64).

Attention core: S = K^T@Q in f32r (1 cyc/row, 64-wide contraction), exp
on Act -> pt bf16, AV transposed (psum[q, 4, 65] += pt_chunk.T @ vh_kb)
in bf16, diag mask-mul on gpsimd (Pool is otherwise idle; DVE is loaded
with evictions).  The S/AV path cannot ride DoubleRow: fp8 quantization
of Q/K/P injects ~3-5% attention-weight noise, over the 2e-2 gate.

ctxT is produced as an fp8 hi/lo pair (DVE quantize + subtract off the
transpose psum) feeding the DR output projection.  Scheduling: flat item
stream with two filler queues — dq holds proj units with just-in-time
deadlines (per m-chunk: qt/kt chunk j lands right before head pair 2j;
V(n) before slice n's first AV drain, which lags 10 items so slice 0's V
can wait out its DMA), sq holds deferred C units popped when the Act
stream is ahead.  dq pops on deadline only: margin-popping would run
ahead of the serialized DMA stream and stall the in-order PE.  x tiles
are whole-slice single-DMA and prefetched a slice ahead; weights are one
DMA each (HWDGE costs ~665ns/DMA, so descriptor count matters).  Last-
slice C psums alternate into the freed S banks and their evictions
alternate Act/DVE to double the tail pipeline.  PSUM: 2x S sets (4
banks) + AV (1) + general (2) + transpose (1) = 8 banks.
TimelineSim: 212,740 ns (prior bf16 kernel: 245,461; rel err 4.6e-3).
"""

import numpy as np
import os as _os

import concourse.bacc as bacc
import concourse.bass as bass
import concourse.mybir as mybir
import concourse.tile as tile
from concourse.bass_utils import run_bass_kernel_spmd

F32 = mybir.dt.float32
F32R = mybir.dt.float32r
BF16 = mybir.dt.bfloat16
F8 = mybir.dt.float8e4
DR = mybir.MatmulPerfMode.DoubleRow

B, L, D, H, DK = 4, 2048, 1024, 16, 64
HD = 8              # heads per core
GW = 512            # head-group width
AUGW = HD * (DK + 1)  # 520
NCH = D // 128      # 8 contraction chunks
SL = 512            # token slice
NS = L // SL        # 4
NKB = L // 128      # 16

PE_NS = 1.0 / 2.4   # ns per PE cycle at full clock
ACT_NS = 1.0 / 1.2  # ns per Act cycle

SQ = 32.0 / (8.0 ** 0.5)   # scale folded into wq/wk before e4m3 split
SV = 32.0                  # scale folded into wv
SONE = 8.0                 # vh ones-column value -> ctx lands at 4*ctx
SO = 16.0                  # scale folded into wo
C_EVICT = 1.0 / 64.0       # (4*ctx)*(16*wo) -> /64
EXP_SCALE = 1.0 / 1024.0   # qt*kt = 1024*S_true


def _build_nc(dbg=False):
    nc = bacc.Bacc("TRN2", target_bir_lowering=False, debug=False, num_devices=8)

    # x/w tensors carry [hi; lo] e4m3 blocks stacked on the row (contraction)
    # axis; host does the split.
    xq = nc.dram_tensor("xq", [2 * D, L], F8, kind="ExternalInput").ap()
    xk = nc.dram_tensor("xk", [2 * D, L], F8, kind="ExternalInput").ap()
    xv = nc.dram_tensor("xv", [2 * D, L], F8, kind="ExternalInput").ap()
    wq = nc.dram_tensor("wq", [2 * D, GW], F8, kind="ExternalInput").ap()
    wk = nc.dram_tensor("wk", [2 * D, GW], F8, kind="ExternalInput").ap()
    wv = nc.dram_tensor("wv", [2 * D, AUGW], F8, kind="ExternalInput").ap()
    wo = nc.dram_tensor("wo", [2 * GW, D], F8, kind="ExternalInput").ap()
    bqk = nc.dram_tensor("bqk", [128, 8], F32, kind="ExternalInput").ap()
    vb = nc.dram_tensor("vb", [AUGW], F32, kind="ExternalInput").ap()
    mi = nc.dram_tensor("mi", [128, 256], BF16, kind="ExternalInput").ap()
    outp = nc.dram_tensor("outp", [L, D], BF16, kind="ExternalOutput").ap()
    if dbg:
        qt_dbg = nc.dram_tensor("qt_dbg", [128, 4 * L], F32, kind="ExternalOutput").ap()
        kt_dbg = nc.dram_tensor("kt_dbg", [128, 4 * L], F32, kind="ExternalOutput").ap()
        vh_dbg = nc.dram_tensor("vh_dbg", [128, NKB * 520], F32,
                                kind="ExternalOutput").ap()
        ctx_dbg = nc.dram_tensor("ctx_dbg", [128, 4 * L], F32,
                                 kind="ExternalOutput").ap()
        av_dbg = nc.dram_tensor("av_dbg", [128, HD * 4 * 65], F32,
                                kind="ExternalOutput").ap()
        ct_dbg = nc.dram_tensor("ct_dbg", [128, HD * 4 * DK], F32,
                                kind="ExternalOutput").ap()
        s_dbg = nc.dram_tensor("s_dbg", [128, HD * 1024], F32,
                               kind="ExternalOutput").ap()
        pt_dbg = nc.dram_tensor("pt_dbg", [128, HD * 1024], F32,
                                kind="ExternalOutput").ap()

    mask_eng_env = _os.environ.get("K_MASKDVE")

    with tile.TileContext(nc) as tc:
        with (
            tc.tile_pool(name="persist", bufs=1) as persist,
            tc.tile_pool(name="xin", bufs=4 if dbg else 6) as xinp,
            tc.tile_pool(name="pt", bufs=10) as ptp,
            tc.tile_pool(name="ctx", bufs=4) as ctxp,
            tc.tile_pool(name="ctxT8", bufs=4) as ctxT8p,
            tc.tile_pool(name="small", bufs=8) as smallp,
            tc.tile_pool(name="outs", bufs=4) as outsp,
            tc.tile_pool(name="psS", bufs=2, space="PSUM") as psS,
            tc.tile_pool(name="psAV", bufs=1, space="PSUM") as psAV,
            tc.tile_pool(name="psG", bufs=1, space="PSUM") as psG,
        ):
            # ---- persistent SBUF ----
            # weights: [128, s(hi/lo), ktile, cols]
            wq_s = persist.tile([128, 2, NCH, GW], F8, tag="wq")
            wk_s = persist.tile([128, 2, NCH, GW], F8, tag="wk")
            wv_s = persist.tile([128, 2, NCH, AUGW], F8, tag="wv")
            wo_s = persist.tile([128, 2, 4, D], F8, tag="wo")
            qt_s = persist.tile([128, 4, L], F32R, tag="qt")
            kt_s = persist.tile([128, 4, L], F32R, tag="kt")
            vh_s = persist.tile([128, NKB, 2, 260], BF16, tag="vh")
            bqk_s = persist.tile([128, 8], F32, tag="bqk")
            vb_s = persist.tile([128, AUGW], F32, tag="vb")
            mi_s = persist.tile([128, 256], BF16, tag="mi")

            def emit_w_dmas(which, split=False):
                if which == "bqk":
                    nc.sync.dma_start(bqk_s[:, :], bqk[:, :])
                elif which == "first":
                    nc.sync.dma_start(mi_s[:, :], mi[:, :])
                    vb_bcast = bass.AP(tensor=vb.tensor, offset=vb.offset,
                                       ap=[[0, 128], [1, AUGW]])
                    nc.sync.dma_start(vb_s[:, :], vb_bcast)
                elif which in ("q", "k", "v"):
                    w_s, w_d = {"q": (wq_s, wq), "k": (wk_s, wk),
                                "v": (wv_s, wv)}[which]
                    src = w_d.rearrange("(s c p) q -> p s c q", s=2, p=128)
                    if split:
                        for s in range(2):
                            nc.sync.dma_start(w_s[:, s, :, :], src[:, s, :, :])
                    else:
                        nc.sync.dma_start(w_s[:, :, :, :], src)
                else:
                    nc.sync.dma_start(
                        wo_s[:, :, :, :],
                        wo.rearrange("(s c p) q -> p s c q", s=2, p=128))

            # ---------- pacing counters (ns, at full clocks) ----------
            st = {"pe": 0.0, "act": 0.0}

            def mm(*args, **kw):
                out = args[0]
                st["pe"] += out.free_size() * PE_NS
                nc.tensor.matmul(*args, **kw)

            def mmdr(*args, **kw):
                out = args[0]
                st["pe"] += out.free_size() * 0.5 * PE_NS
                nc.tensor.matmul(*args, perf_mode=DR, **kw)

            # ---------- projection / output-projection units ----------
            def emit_x_dma(n, src, tag, split=False):
                # whole-slice tile [128, s(hi/lo), 8 ktiles, SL], one DMA
                # (two when split: hi first so class-A matmuls start earlier)
                t = xinp.tile([128, 2, NCH, SL], F8, tag="x", name=f"x{tag}{n}")
                s_ap = src.rearrange("(s c p) q -> p s c q", s=2, p=128)[
                    :, :, :, n * SL:(n + 1) * SL]
                if split:
                    for s in range(2):
                        nc.sync.dma_start(t[:, s, :, :], s_ap[:, s, :, :])
                else:
                    nc.sync.dma_start(t[:, :, :, :], s_ap)
                return t

            # 3-slot fp8 classes: (ws, xs) in A=(hi,hi), B=(lo,hi), C=(hi,lo)
            SLOT3 = ((0, 0), (1, 0), (0, 1))

            def emit_qk_unit(n, xt, w_s, dst, b_s, m, ci, psh):
                # ci: slot-class index 0..2 (emission granularity), or None
                cis = range(3) if ci is None else (ci,)
                if cis[0] == 0:
                    psh[m] = psG.tile([128, 512], F32, tag="g", bufs=2,
                                      name=f"qk{n}_{m}")
                ps = psh[m]
                for c3 in cis:
                    ws, xs = SLOT3[c3]
                    for cp in range(4):
                        cg = cp * 2
                        mmdr(ps[:, :],
                             w_s[:, ws, cg:cg + 2, m * 128:(m + 1) * 128],
                             xt[:, xs, cg:cg + 2, :],
                             start=(c3 == 0 and cp == 0),
                             stop=(c3 == 2 and cp == 3))
                if cis[-1] == 2:
                    del psh[m]
                    nc.vector.tensor_scalar_add(
                        dst[:, m, n * SL:(n + 1) * SL], ps[:, :],
                        b_s[:, m:m + 1])

            def emit_v_unit(n, xt, tt, hf, ci, psh):
                cis = range(3) if ci is None else (ci,)
                if cis[0] == 0:
                    psh[(tt, hf)] = psG.tile([128, 512], F32, tag="g", bufs=2,
                                             name=f"v{n}_{tt}_{hf}")
                ps = psh[(tt, hf)]
                for c3 in cis:
                    xs, ws = SLOT3[c3]  # lhsT is x here
                    for cp in range(4):
                        cg = cp * 2
                        mmdr(ps[:, 0:260],
                             xt[:, xs, cg:cg + 2, tt * 128:(tt + 1) * 128],
                             wv_s[:, ws, cg:cg + 2,
                                  hf * 260:(hf + 1) * 260],
                             start=(c3 == 0 and cp == 0),
                             stop=(c3 == 2 and cp == 3))
                if cis[-1] == 2:
                    del psh[(tt, hf)]
                    kb = n * 4 + tt
                    nc.vector.tensor_add(
                        vh_s[:, kb, hf, :],
                        ps[:, 0:260], vb_s[:, hf * 260:(hf + 1) * 260])

            def emit_c_unit(n, tt, n2, ctxT8_n):
                if n >= NS - 1 and (tt + n2) % 2 == 0:
                    # the S psum banks are free once the last exps retire;
                    # alternating pools doubles the tail eviction pipeline
                    ps2 = psS.tile([128, 2, 512], F32, tag="s",
                                   name=f"c{n}_{tt}_{n2}")
                    ps = ps2[:, 0, :]
                else:
                    ps = psG.tile([128, 512], F32, tag="g", bufs=2,
                                  name=f"c{n}_{tt}_{n2}")
                i = 0
                for s_ctx, s_wo in SLOT3:
                    for cp in range(2):
                        mmdr(ps[:, :],
                             ctxT8_n[:, s_ctx, cp * 2:cp * 2 + 2,
                                     tt * 128:(tt + 1) * 128],
                             wo_s[:, s_wo, cp * 2:cp * 2 + 2,
                                  n2 * 512:(n2 + 1) * 512],
                             start=(i == 0), stop=(i == 5))
                        i += 1
                ot = outsp.tile([128, 512], BF16, tag="ot", name=f"ot{n}_{tt}_{n2}")
                if n >= NS - 1:
                    # Act is done with exps by the time these pop; DVE still
                    # has the last ctxT8 hi/lo work queued
                    nc.scalar.activation(ot[:, :], ps[:, :],
                                         func=mybir.ActivationFunctionType.Copy,
                                         scale=C_EVICT)
                else:
                    nc.vector.tensor_scalar_mul(ot[:, :], ps[:, :], C_EVICT)
                nc.sync.dma_start(
                    outp[(n * 4 + tt) * 128:(n * 4 + tt + 1) * 128,
                         n2 * 512:(n2 + 1) * 512], ot[:, :])

            def c_units(n, ctxT8_n):
                units = []
                for tt in range(4):
                    for n2 in range(2):
                        units.append(lambda n=n, tt=tt, n2=n2: emit_c_unit(
                            n, tt, n2, ctxT8_n))
                return units

            # ---------- prologue: DMAs + Q/K m0 only (act starts ASAP) ----
            # interleaved hi-first DMA order so class-A DR matmuls can start
            # after ~1MB of transfer; the small DMAs queue behind the first
            # consumer-critical MBs (each dma_start costs ~665ns of HWDGE):
            # bqk before the first qt evict, mi/vb before the first mask/V
            # evict (not needed until drain item 10 / V pops)
            wq_src = wq.rearrange("(s c p) q -> p s c q", s=2, p=128)
            wk_src = wk.rearrange("(s c p) q -> p s c q", s=2, p=128)
            xts = {}
            xts["q"] = xinp.tile([128, 2, NCH, SL], F8, tag="x", name="xq0")
            xts["k"] = xinp.tile([128, 2, NCH, SL], F8, tag="x", name="xk0")
            xq_src = xq.rearrange("(s c p) q -> p s c q", s=2, p=128)
            xk_src = xk.rearrange("(s c p) q -> p s c q", s=2, p=128)
            for s in range(2):
                nc.sync.dma_start(wq_s[:, s, :, :], wq_src[:, s, :, :])
                nc.sync.dma_start(xts["q"][:, s, :, :], xq_src[:, s, :, 0:SL])
            emit_w_dmas("bqk")
            for s in range(2):
                nc.sync.dma_start(wk_s[:, s, :, :], wk_src[:, s, :, :])
                nc.sync.dma_start(xts["k"][:, s, :, :], xk_src[:, s, :, 0:SL])
            emit_w_dmas("first")
            # full QK(0): the later m-chunks overlap the wk/xk DMA waits
            psh0 = {}
            for m in range(4):
                emit_qk_unit(0, xts["q"], wq_s, qt_s, bqk_s[:, 0:4], m, None, psh0)
            for m in range(4):
                emit_qk_unit(0, xts["k"], wk_s, kt_s, bqk_s[:, 4:8], m, None, psh0)
            emit_w_dmas("v")
            xts["v"] = emit_x_dma(0, xv, "v")

            # ---------- attention + pipeline ----------
            ctxT8_all = {}   # n -> [128, 2, 4, 512] fp8 hi/lo tile

            # Two filler queues paced against the Act exp stream:
            #  - dq: hard-deadline entries (due, thunk), FIFO in due order —
            #    projection units gated just-in-time per m-chunk / slice
            #  - sq: soft entries (deferred C units), popped on margin only
            dq = []
            sq = []
            margin = float(_os.environ.get("K_MARGIN", 2500))

            def pop_fillers(now=None, force_all=False):
                # dq pops strictly at deadline (margin pops would run ahead
                # of the DMA stream and stall the in-order PE); sq (C units)
                # absorbs the margin slack
                while dq and (force_all
                              or (now is not None and dq[0][0] <= now)):
                    dq.pop(0)[1]()
                while sq and (force_all or st["pe"] < st["act"] + margin):
                    sq.pop(0)()

            # prologue PE work ran concurrent with the x/w DMA stream; start
            # the pacing race fresh at the item stream
            st["act"] = st["pe"]

            def gidx(n, ii):
                return n * 1000 + ii

            def queue_qk_m(n1, m, xtd, psh, n_due, ii_due):
                # spread: 2 entries per item starting at (n_due, ii_due)
                ents = []
                for ci in range(3):
                    ents.append(lambda m=m, ci=ci: emit_qk_unit(
                        n1, xtd["q"], wq_s, qt_s, bqk_s[:, 0:4], m, ci, psh))
                for ci in range(3):
                    ents.append(lambda m=m, ci=ci: emit_qk_unit(
                        n1, xtd["k"], wk_s, kt_s, bqk_s[:, 4:8], m, ci, psh))
                for i, e in enumerate(ents):
                    dq.append((gidx(n_due, ii_due + i // 2), e))

            def queue_v(n1, xtd, psh, n_due, ii_due, spread=2):
                # AV consumes the diagonal key blocks (high tt) first and
                # heads 0-3 (hf=0) before 4-7
                ents = [lambda tt=tt, hf=hf: emit_v_unit(
                            n1, xtd["v"], tt, hf, None, psh)
                        for tt in range(4) for hf in range(2)]
                for i, e in enumerate(ents):
                    dq.append((gidx(n_due, ii_due + i // spread), e))

            # V(0) just-in-time within slice 0 (after its DMA lands); wo
            # rides the queue behind the slice-1 x prefetch
            # V(0) pops at items 5-8: after the wv/xv0 DMAs land (no in-order
            # PE stall) and before the first AV drain (at item 10)
            psh0b = {}
            queue_v(0, xts, psh0b, 0, 5)
            dq.append((gidx(0, 9), lambda: emit_w_dmas("rest")))

            for n in range(NS):
                if n + 1 < NS:
                    # eager whole-slice x prefetch + JIT-gated proj units;
                    # V(n+1)+m0(n+1) spread over the tail items of slice n
                    n1 = n + 1
                    nxts = {"q": emit_x_dma(n1, xq, "q"),
                            "k": emit_x_dma(n1, xk, "k"),
                            "v": emit_x_dma(n1, xv, "v")}
                    pshn = {}
                    ni = HD * 2 * (n + 1)    # items in slice n
                    npg = 2 * (n1 + 1)       # items per head in slice n1
                    queue_v(n1, nxts, pshn, n, ni - 7)
                    queue_qk_m(n1, 0, nxts, pshn, n, ni - 3)
                    for j in (1, 2, 3):
                        queue_qk_m(n1, j, nxts, pshn,
                                   n1, max(0, 2 * j * npg - 7))

                nkb = 4 * n + 4
                ngrp = nkb // 2
                ctxT8_n = ctxT8p.tile([128, 2, 4, 512], F8, tag="ctxT8",
                                      name=f"ctxT8_{n}")
                ctxT8_all[n] = ctxT8_n
                psT_cur = [None]

                def emit_s_exp(h, g, pts):
                    # Both banks of a group share the group's column base so a
                    # single 2-bank exp covers them (the extra computed scores
                    # land in q-chunks the AV stage never reads).
                    mc, po = h // 2, (h % 2) * 64
                    sps = psS.tile([128, 2, 512], F32, tag="s",
                                   name=f"s{n}_{h}_{g}")
                    pt = ptp.tile([128, 2, 512], BF16, tag="pt",
                                  name=f"pt{n}_{h}_{g}")
                    c0a = max(0, 2 * g * 128 - n * SL)
                    for i in range(2):
                        kb = 2 * g + i
                        # per-kb trim: queries before the key block are dead
                        # (AV skips them); exp still reads from c0a, the
                        # stale-psum cols it covers land in dead pt slots.
                        # f32r needs >=256 moving cols for 1 cyc/row.
                        c0i = min(max(0, kb * 128 - n * SL), SL - 256)
                        mm(sps[:, i, c0i:],
                           kt_s[po:po + 64, mc, kb * 128:(kb + 1) * 128],
                           qt_s[po:po + 64, mc, n * SL + c0i:(n + 1) * SL],
                           start=True, stop=True)
                    if dbg and n == 0 and g == 0:
                        sd = outsp.tile([128, 2, 512], F32, tag="sd", bufs=1,
                                        name=f"sd{n}_{h}_{g}")
                        nc.vector.tensor_copy(sd[:, :, :], sps[:, :, :])
                        nc.sync.dma_start(
                            s_dbg[:, h * 1024:(h + 1) * 1024], sd[:, :, :])
                    st["act"] += (2 * (512 - c0a)) * ACT_NS + 185.0
                    nc.scalar.activation(
                        pt[:, :, c0a:], sps[:, :, c0a:],
                        func=mybir.ActivationFunctionType.Exp,
                        scale=EXP_SCALE)
                    if dbg and n == 0 and g == 0:
                        pd = outsp.tile([128, 2, 512], F32, tag="pd", bufs=1,
                                        name=f"pd{n}_{h}_{g}")
                        nc.vector.tensor_copy(pd[:, :, :], pt[:, :, :])
                        nc.sync.dma_start(
                            pt_dbg[:, h * 1024:(h + 1) * 1024], pd[:, :, :])
                    pts[(h, g)] = pt

                def emit_av(h, g, avp, pts, first_grp, last_grp):
                    # PSUM start=True lazily zero-marks the WHOLE bank, so
                    # only the first emitted write into the bank may use it;
                    # later first-writes per region overwrite via the
                    # pending-zero flags.  Accumulation order over kb is free.
                    pt = pts.pop((h, g))
                    for i in range(2):
                        kb = 2 * g + i
                        if kb >= 4 * n:  # diagonal block: causal mask
                            col0 = max(0, kb * 128 - n * SL)
                            (nc.vector if mask_eng_env
                             else nc.gpsimd).tensor_mul(
                                pt[:, i, col0:col0 + 128],
                                pt[:, i, col0:col0 + 128], mi_s[:, 0:128])
                    started = [not (g == first_grp)]
                    for i in range(2):
                        kb = 2 * g + i
                        for qc in range(4):
                            if kb > 4 * n + qc:
                                continue
                            last = (g == last_grp) and (
                                kb == min(1, 4 * n + qc))
                            mm(avp[:, qc, 0:65],
                               pt[:, i, qc * 128:(qc + 1) * 128],
                               vh_s[:, kb, h // 4, (h % 4) * 65:(h % 4) * 65 + 65],
                               start=not started[0],
                               stop=last,
                               skip_group_check=True)
                            started[0] = True

                def emit_tail(h, avp):
                    # normalize: ctx_t[q, d] = 4 * av[q, d] / (av[q, 64]/8)
                    mc, po = h // 2, (h % 2) * 64
                    rcp = smallp.tile([128, 4, 1], F32, tag="rcp",
                                      name=f"rcp{n}_{h}")
                    nc.vector.reciprocal(rcp[:, :], avp[:, :, 64:65])
                    ctx_t = ctxp.tile([128, 4, DK], BF16, tag="ctx",
                                      name=f"ctx{n}_{h}")
                    if dbg and n == 0:
                        avd = outsp.tile([128, 4, 65], F32, tag="avd", bufs=1,
                                         name=f"avd{n}_{h}")
                        nc.vector.tensor_copy(avd[:, :, :], avp[:, :, :65])
                        nc.sync.dma_start(
                            av_dbg[:, h * 260:(h + 1) * 260], avd[:, :, :])
                    for qc in range(4):
                        nc.vector.tensor_scalar_mul(
                            ctx_t[:, qc, :], avp[:, qc, 0:DK], rcp[:, qc, :])
                    if dbg and n == 0:
                        ctd = outsp.tile([128, 4, DK], F32, tag="ctd", bufs=1,
                                         name=f"ctd{n}_{h}")
                        nc.vector.tensor_copy(ctd[:, :, :], ctx_t[:, :, :])
                        nc.sync.dma_start(
                            ct_dbg[:, h * 256:(h + 1) * 256], ctd[:, :, :])
                    # transpose to dims-major; 2 heads share one psum tile
                    if h % 2 == 0:
                        psT_cur[0] = psG.tile([128, 512], BF16, tag="t",
                                              name=f"t{n}_{mc}")
                    psT = psT_cur[0]
                    for qc in range(4):
                        st["pe"] += 128 * PE_NS
                        nc.tensor.transpose(
                            psT[po:po + 64, qc * 128:(qc + 1) * 128],
                            ctx_t[:, qc, :], mi_s[:, 128:256])
                    if h % 2 == 1:
                        # fp8 hi/lo split of the transposed ctx chunk
                        nc.vector.tensor_copy(ctxT8_n[:, 0, mc, :], psT[:, :])
                        nc.vector.tensor_sub(ctxT8_n[:, 1, mc, :], psT[:, :],
                                             ctxT8_n[:, 0, mc, :])

                # flat (head, group) pipeline: AV lags S/exp by one item so
                # head boundaries don't bunch the Act queue against psS WARs
                gorder = list(range(ngrp - 1, -1, -1))  # diag groups first
                items = [(h, g) for h in range(HD) for g in gorder]
                pts = {}
                avps = {}
                pending = []

                def drain_av():
                    ph, pg = pending.pop(0)
                    emit_av(ph, pg, avps[ph], pts, gorder[0], gorder[-1])
                    if pg == gorder[-1]:
                        emit_tail(ph, avps.pop(ph))

                for ii, (h, g) in enumerate(items):
                    if g == gorder[0]:
                        avps[h] = psAV.tile([128, 4, 128], F32, tag="av",
                                            name=f"av{n}_{h}")
                    emit_s_exp(h, g, pts)
                    pending.append((h, g))
                    # deep lag lets slice-0's V wait out its DMA before the
                    # first AV; the last slice drains shallow so its tails
                    # (and so the C(3) units) finish earlier — there the S
                    # stream is already act-paced, so AV never waits on pt
                    if len(pending) > (9 if n < NS - 1 else 2):
                        drain_av()
                    pop_fillers(now=gidx(n, ii))
                while pending:
                    drain_av()

                # C units for this slice become available now
                sq.extend(c_units(n, ctxT8_n))
                if dbg:
                    for c in range(4):
                        ct = outsp.tile([128, 512], F32, tag="dbg", bufs=1,
                                        name=f"cdb{n}_{c}")
                        h8 = outsp.tile([128, 512], F32, tag="dbg8", bufs=1,
                                        name=f"cdb8{n}_{c}")
                        nc.vector.tensor_copy(h8[:, :], ctxT8_n[:, 0, c, :])
                        nc.vector.tensor_add(ct[:, :], h8[:, :],
                                             ctxT8_n[:, 1, c, :])
                        nc.sync.dma_start(
                            ctx_dbg[:, (n * 4 + c) * 512:(n * 4 + c + 1) * 512],
                            ct[:, :])

            # ---------- drain remaining fillers (incl. all deferred C) ----
            pop_fillers(force_all=True)

            if dbg:
                nc.sync.dma_start(qt_dbg[:, :], qt_s[:, :, :].bitcast(F32))
                nc.sync.dma_start(kt_dbg[:, :], kt_s[:, :, :].bitcast(F32))
                for kb in range(NKB):
                    for hf in range(2):
                        vt = outsp.tile([128, 512], F32, tag="dbg", bufs=1,
                                        name=f"vdb{kb}_{hf}")
                        nc.vector.tensor_copy(vt[:, 0:260], vh_s[:, kb, hf, :])
                        nc.sync.dma_start(
                            vh_dbg[:, kb * 520 + hf * 260:
                                   kb * 520 + (hf + 1) * 260], vt[:, 0:260])

    nc.compile()
    return nc


_NC = None
LAST_RESULTS = None


def _hl(x, f8):
    h = x.astype(f8)
    l = (x - h.astype(np.float32)).astype(f8)
    return np.concatenate([h, l], axis=0)


def kernel(**inputs):
    global _NC, LAST_RESULTS
    import os
    import ml_dtypes
    if _NC is None:
        _NC = _build_nc()

    f8 = ml_dtypes.float8_e4m3
    bf = ml_dtypes.bfloat16
    f = lambda a: np.asarray(a, dtype=np.float32)
    q, k, v = f(inputs["q"]), f(inputs["k"]), f(inputs["v"])
    wq_w, wq_b = f(inputs["wq_w"]), f(inputs["wq_b"])
    wk_w, wk_b = f(inputs["wk_w"]), f(inputs["wk_b"])
    wv_w, wv_b = f(inputs["wv_w"]), f(inputs["wv_b"])
    wo_w, wo_b = f(inputs["wo_w"]), f(inputs["wo_b"])

    msk = np.ascontiguousarray(
        (np.arange(128)[None, :] >= np.arange(128)[:, None])).astype(bf)
    idn = np.eye(128).astype(bf)
    mi = np.ascontiguousarray(np.concatenate([msk, idn], axis=1))

    gmaps = []
    for g in range(2):
        sl = slice(g * GW, (g + 1) * GW)
        wq8 = _hl(np.ascontiguousarray(wq_w[sl].T * SQ), f8)
        wk8 = _hl(np.ascontiguousarray(wk_w[sl].T * SQ), f8)
        wvT = np.zeros((D, AUGW), np.float32)
        vbias = np.zeros((AUGW,), np.float32)
        for h in range(HD):
            wvT[:, h * 65:h * 65 + 64] = wv_w[g * GW + h * 64:
                                              g * GW + (h + 1) * 64].T * SV
            vbias[h * 65:h * 65 + 64] = wv_b[g * GW + h * 64:
                                             g * GW + (h + 1) * 64] * SV
            vbias[h * 65 + 64] = SONE
        wv8 = _hl(wvT, f8)
        wo8 = _hl(np.ascontiguousarray(wo_w[:, sl].T * SO), f8)
        bqT = np.ascontiguousarray((wq_b[sl] * SQ).reshape(4, 128).T)
        bkT = np.ascontiguousarray((wk_b[sl] * SQ).reshape(4, 128).T)
        gmaps.append(dict(wq=wq8, wk=wk8, wv=wv8, wo=wo8,
                          bqk=np.ascontiguousarray(
                              np.concatenate([bqT, bkT], axis=1)),
                          vb=vbias, mi=mi))

    bmaps = []
    for b in range(B):
        bmaps.append(dict(
            xq=_hl(np.ascontiguousarray(q[b].T), f8),
            xk=_hl(np.ascontiguousarray(k[b].T), f8),
            xv=_hl(np.ascontiguousarray(v[b].T), f8)))

    in_maps = [dict(**bmaps[c // 2], **gmaps[c % 2]) for c in range(8)]

    trace = bool(int(os.environ.get("KERNEL_TRACE", "0")))
    res = run_bass_kernel_spmd(_NC, in_maps, list(range(8)), trace=trace)
    LAST_RESULTS = res

    out = np.empty((B, L, D), np.float32)
    for b in range(B):
        out[b] = (np.asarray(res.results[2 * b]["outp"], np.float32)
                  + np.asarray(res.results[2 * b + 1]["outp"], np.float32)
                  + wo_b[None, :])
    return out


# revision 57
# speedup vs baseline: 1.0069x; 1.0022x over previous
"""Causal MHA (B=4, L=2048, D=1024, H=16) on 8 NeuronCores — fused pipeline.

Sharding: core c -> (batch b = c//2, head-group g = c%2), 8 heads/core.
wq/wk/wv column-parallel, wo row-parallel; host sums the two half-group
partials per batch and adds wo_b.

Single dataflow pipeline per core.  All four projection-class matmuls
(Q/K/V proj and the output projection C) run in fp8e4 DoubleRow perf mode
(0.5 PE cycles/row, contraction 2x128 per instr).  Precision is held at
~bf16 level with a 3-slot hi/lo decomposition: operand a = a_hi + a_lo
(both e4m3, host-split), product = ah*bh + al*bh + ah*bl (lo*lo dropped),
so each 8-ktile contraction costs 12 DR instrs = 6N cycles vs bf16's 8N.
Operands are pre-scaled by powers of 2 into e4m3's normal range; the
rescales fold into the exp scale (S arrives as 1024*S, exp applies
scale=1/1024), the vh ones-column (denominator carries the V scale so
ctx = num*rcp lands at 4*ctx, e4m3-ranged), and the C eviction (1/64).

Attention core: S = K^T@Q in f32r (1 cyc/row, 64-wide contraction), exp
on Act -> pt bf16, AV transposed (psum[q, 4, 65] += pt_chunk.T @ vh_kb)
in bf16, diag mask-mul on gpsimd (Pool is otherwise idle; DVE is loaded
with evictions).  The S/AV path cannot ride DoubleRow: fp8 quantization
of Q/K/P injects ~3-5% attention-weight noise, over the 2e-2 gate.

ctxT is produced as an fp8 hi/lo pair (DVE quantize + subtract off the
transpose psum) feeding the DR output projection.  Scheduling: flat item
stream with two filler queues — dq holds proj units with just-in-time
deadlines (per m-chunk: qt/kt chunk j lands right before head pair 2j;
V(n) before slice n's first AV drain, which lags 10 items so slice 0's V
can wait out its DMA), sq holds deferred C units popped when the Act
stream is ahead.  dq pops on deadline only: margin-popping would run
ahead of the serialized DMA stream and stall the in-order PE.  x tiles
are whole-slice single-DMA and prefetched a slice ahead; weights are one
DMA each (HWDGE costs ~665ns/DMA, so descriptor count matters).  Last-
slice C psums alternate into the freed S banks and their evictions
alternate Act/DVE to double the tail pipeline.  PSUM: 2x S sets (4
banks) + AV (1) + general (2) + transpose (1) = 8 banks.
TimelineSim: 211,808 ns (prior bf16 kernel: 245,461; rel err 4.6e-3).
"""

import numpy as np
import os as _os

import concourse.bacc as bacc
import concourse.bass as bass
import concourse.mybir as mybir
import concourse.tile as tile
from concourse.bass_utils import run_bass_kernel_spmd

F32 = mybir.dt.float32
F32R = mybir.dt.float32r
BF16 = mybir.dt.bfloat16
F8 = mybir.dt.float8e4
DR = mybir.MatmulPerfMode.DoubleRow

B, L, D, H, DK = 4, 2048, 1024, 16, 64
HD = 8              # heads per core
GW = 512            # head-group width
AUGW = HD * (DK + 1)  # 520
NCH = D // 128      # 8 contraction chunks
SL = 512            # token slice
NS = L // SL        # 4
NKB = L // 128      # 16

PE_NS = 1.0 / 2.4   # ns per PE cycle at full clock
ACT_NS = 1.0 / 1.2  # ns per Act cycle

SQ = 32.0 / (8.0 ** 0.5)   # scale folded into wq/wk before e4m3 split
SV = 32.0                  # scale folded into wv
SONE = 8.0                 # vh ones-column value -> ctx lands at 4*ctx
SO = 16.0                  # scale folded into wo
C_EVICT = 1.0 / 64.0       # (4*ctx)*(16*wo) -> /64
EXP_SCALE = 1.0 / 1024.0   # qt*kt = 1024*S_true


def _build_nc(dbg=False):
    nc = bacc.Bacc("TRN2", target_bir_lowering=False, debug=False, num_devices=8)

    # x/w tensors carry [hi; lo] e4m3 blocks stacked on the row (contraction)
    # axis; host does the split.
    xq = nc.dram_tensor("xq", [2 * D, L], F8, kind="ExternalInput").ap()
    xk = nc.dram_tensor("xk", [2 * D, L], F8, kind="ExternalInput").ap()
    xv = nc.dram_tensor("xv", [2 * D, L], F8, kind="ExternalInput").ap()
    wq = nc.dram_tensor("wq", [2 * D, GW], F8, kind="ExternalInput").ap()
    wk = nc.dram_tensor("wk", [2 * D, GW], F8, kind="ExternalInput").ap()
    wv = nc.dram_tensor("wv", [2 * D, AUGW], F8, kind="ExternalInput").ap()
    wo = nc.dram_tensor("wo", [2 * GW, D], F8, kind="ExternalInput").ap()
    bqk = nc.dram_tensor("bqk", [128, 8], F32, kind="ExternalInput").ap()
    vb = nc.dram_tensor("vb", [AUGW], F32, kind="ExternalInput").ap()
    mi = nc.dram_tensor("mi", [128, 256], BF16, kind="ExternalInput").ap()
    outp = nc.dram_tensor("outp", [L, D], BF16, kind="ExternalOutput").ap()
    if dbg:
        qt_dbg = nc.dram_tensor("qt_dbg", [128, 4 * L], F32, kind="ExternalOutput").ap()
        kt_dbg = nc.dram_tensor("kt_dbg", [128, 4 * L], F32, kind="ExternalOutput").ap()
        vh_dbg = nc.dram_tensor("vh_dbg", [128, NKB * 520], F32,
                                kind="ExternalOutput").ap()
        ctx_dbg = nc.dram_tensor("ctx_dbg", [128, 4 * L], F32,
                                 kind="ExternalOutput").ap()
        av_dbg = nc.dram_tensor("av_dbg", [128, HD * 4 * 65], F32,
                                kind="ExternalOutput").ap()
        ct_dbg = nc.dram_tensor("ct_dbg", [128, HD * 4 * DK], F32,
                                kind="ExternalOutput").ap()
        s_dbg = nc.dram_tensor("s_dbg", [128, HD * 1024], F32,
                               kind="ExternalOutput").ap()
        pt_dbg = nc.dram_tensor("pt_dbg", [128, HD * 1024], F32,
                                kind="ExternalOutput").ap()

    mask_eng_env = _os.environ.get("K_MASKDVE")

    with tile.TileContext(nc) as tc:
        with (
            tc.tile_pool(name="persist", bufs=1) as persist,
            tc.tile_pool(name="xin", bufs=4 if dbg else 6) as xinp,
            tc.tile_pool(name="pt", bufs=10) as ptp,
            tc.tile_pool(name="ctx", bufs=4) as ctxp,
            tc.tile_pool(name="ctxT8", bufs=4) as ctxT8p,
            tc.tile_pool(name="small", bufs=8) as smallp,
            tc.tile_pool(name="outs", bufs=4) as outsp,
            tc.tile_pool(name="psS", bufs=2, space="PSUM") as psS,
            tc.tile_pool(name="psAV", bufs=1, space="PSUM") as psAV,
            tc.tile_pool(name="psG", bufs=1, space="PSUM") as psG,
        ):
            # ---- persistent SBUF ----
            # weights: [128, s(hi/lo), ktile, cols]
            wq_s = persist.tile([128, 2, NCH, GW], F8, tag="wq")
            wk_s = persist.tile([128, 2, NCH, GW], F8, tag="wk")
            wv_s = persist.tile([128, 2, NCH, AUGW], F8, tag="wv")
            wo_s = persist.tile([128, 2, 4, D], F8, tag="wo")
            qt_s = persist.tile([128, 4, L], F32R, tag="qt")
            kt_s = persist.tile([128, 4, L], F32R, tag="kt")
            vh_s = persist.tile([128, NKB, 2, 260], BF16, tag="vh")
            bqk_s = persist.tile([128, 8], F32, tag="bqk")
            vb_s = persist.tile([128, AUGW], F32, tag="vb")
            mi_s = persist.tile([128, 256], BF16, tag="mi")

            def emit_w_dmas(which, split=False):
                if which == "bqk":
                    nc.sync.dma_start(bqk_s[:, :], bqk[:, :])
                elif which == "first":
                    nc.sync.dma_start(mi_s[:, :], mi[:, :])
                    vb_bcast = bass.AP(tensor=vb.tensor, offset=vb.offset,
                                       ap=[[0, 128], [1, AUGW]])
                    nc.sync.dma_start(vb_s[:, :], vb_bcast)
                elif which in ("q", "k", "v"):
                    w_s, w_d = {"q": (wq_s, wq), "k": (wk_s, wk),
                                "v": (wv_s, wv)}[which]
                    src = w_d.rearrange("(s c p) q -> p s c q", s=2, p=128)
                    if split:
                        for s in range(2):
                            nc.sync.dma_start(w_s[:, s, :, :], src[:, s, :, :])
                    else:
                        nc.sync.dma_start(w_s[:, :, :, :], src)
                else:
                    nc.sync.dma_start(
                        wo_s[:, :, :, :],
                        wo.rearrange("(s c p) q -> p s c q", s=2, p=128))

            # ---------- pacing counters (ns, at full clocks) ----------
            st = {"pe": 0.0, "act": 0.0}

            def mm(*args, **kw):
                out = args[0]
                st["pe"] += out.free_size() * PE_NS
                nc.tensor.matmul(*args, **kw)

            def mmdr(*args, **kw):
                out = args[0]
                st["pe"] += out.free_size() * 0.5 * PE_NS
                nc.tensor.matmul(*args, perf_mode=DR, **kw)

            # ---------- projection / output-projection units ----------
            def emit_x_dma(n, src, tag, split=False):
                # whole-slice tile [128, s(hi/lo), 8 ktiles, SL], one DMA
                # (two when split: hi first so class-A matmuls start earlier)
                t = xinp.tile([128, 2, NCH, SL], F8, tag="x", name=f"x{tag}{n}")
                s_ap = src.rearrange("(s c p) q -> p s c q", s=2, p=128)[
                    :, :, :, n * SL:(n + 1) * SL]
                if split:
                    for s in range(2):
                        nc.sync.dma_start(t[:, s, :, :], s_ap[:, s, :, :])
                else:
                    nc.sync.dma_start(t[:, :, :, :], s_ap)
                return t

            # 3-slot fp8 classes: (ws, xs) in A=(hi,hi), B=(lo,hi), C=(hi,lo)
            SLOT3 = ((0, 0), (1, 0), (0, 1))

            def emit_qk_unit(n, xt, w_s, dst, b_s, m, ci, psh):
                # ci: slot-class index 0..2 (emission granularity), or None
                cis = range(3) if ci is None else (ci,)
                if cis[0] == 0:
                    psh[m] = psG.tile([128, 512], F32, tag="g", bufs=2,
                                      name=f"qk{n}_{m}")
                ps = psh[m]
                for c3 in cis:
                    ws, xs = SLOT3[c3]
                    for cp in range(4):
                        cg = cp * 2
                        mmdr(ps[:, :],
                             w_s[:, ws, cg:cg + 2, m * 128:(m + 1) * 128],
                             xt[:, xs, cg:cg + 2, :],
                             start=(c3 == 0 and cp == 0),
                             stop=(c3 == 2 and cp == 3))
                if cis[-1] == 2:
                    del psh[m]
                    nc.vector.tensor_scalar_add(
                        dst[:, m, n * SL:(n + 1) * SL], ps[:, :],
                        b_s[:, m:m + 1])

            def emit_v_unit(n, xt, tt, hf, ci, psh):
                cis = range(3) if ci is None else (ci,)
                if cis[0] == 0:
                    psh[(tt, hf)] = psG.tile([128, 512], F32, tag="g", bufs=2,
                                             name=f"v{n}_{tt}_{hf}")
                ps = psh[(tt, hf)]
                for c3 in cis:
                    xs, ws = SLOT3[c3]  # lhsT is x here
                    for cp in range(4):
                        cg = cp * 2
                        mmdr(ps[:, 0:260],
                             xt[:, xs, cg:cg + 2, tt * 128:(tt + 1) * 128],
                             wv_s[:, ws, cg:cg + 2,
                                  hf * 260:(hf + 1) * 260],
                             start=(c3 == 0 and cp == 0),
                             stop=(c3 == 2 and cp == 3))
                if cis[-1] == 2:
                    del psh[(tt, hf)]
                    kb = n * 4 + tt
                    nc.vector.tensor_add(
                        vh_s[:, kb, hf, :],
                        ps[:, 0:260], vb_s[:, hf * 260:(hf + 1) * 260])

            def emit_c_unit(n, tt, n2, ctxT8_n):
                if n >= NS - 1 and (tt + n2) % 2 == 0:
                    # the S psum banks are free once the last exps retire;
                    # alternating pools doubles the tail eviction pipeline
                    ps2 = psS.tile([128, 2, 512], F32, tag="s",
                                   name=f"c{n}_{tt}_{n2}")
                    ps = ps2[:, 0, :]
                else:
                    ps = psG.tile([128, 512], F32, tag="g", bufs=2,
                                  name=f"c{n}_{tt}_{n2}")
                i = 0
                for s_ctx, s_wo in SLOT3:
                    for cp in range(2):
                        mmdr(ps[:, :],
                             ctxT8_n[:, s_ctx, cp * 2:cp * 2 + 2,
                                     tt * 128:(tt + 1) * 128],
                             wo_s[:, s_wo, cp * 2:cp * 2 + 2,
                                  n2 * 512:(n2 + 1) * 512],
                             start=(i == 0), stop=(i == 5))
                        i += 1
                ot = outsp.tile([128, 512], BF16, tag="ot", name=f"ot{n}_{tt}_{n2}")
                if n >= NS - 1:
                    # Act is done with exps by the time these pop; DVE still
                    # has the last ctxT8 hi/lo work queued
                    nc.scalar.activation(ot[:, :], ps[:, :],
                                         func=mybir.ActivationFunctionType.Copy,
                                         scale=C_EVICT)
                else:
                    nc.vector.tensor_scalar_mul(ot[:, :], ps[:, :], C_EVICT)
                nc.sync.dma_start(
                    outp[(n * 4 + tt) * 128:(n * 4 + tt + 1) * 128,
                         n2 * 512:(n2 + 1) * 512], ot[:, :])

            def c_units(n, ctxT8_n):
                units = []
                for tt in range(4):
                    for n2 in range(2):
                        units.append(lambda n=n, tt=tt, n2=n2: emit_c_unit(
                            n, tt, n2, ctxT8_n))
                return units

            # ---------- prologue: DMAs + Q/K m0 only (act starts ASAP) ----
            # interleaved hi-first DMA order so class-A DR matmuls can start
            # after ~1MB of transfer; the small DMAs queue behind the first
            # consumer-critical MBs (each dma_start costs ~665ns of HWDGE):
            # bqk before the first qt evict, mi/vb before the first mask/V
            # evict (not needed until drain item 10 / V pops)
            wq_src = wq.rearrange("(s c p) q -> p s c q", s=2, p=128)
            wk_src = wk.rearrange("(s c p) q -> p s c q", s=2, p=128)
            xts = {}
            xts["q"] = xinp.tile([128, 2, NCH, SL], F8, tag="x", name="xq0")
            xts["k"] = xinp.tile([128, 2, NCH, SL], F8, tag="x", name="xk0")
            xq_src = xq.rearrange("(s c p) q -> p s c q", s=2, p=128)
            xk_src = xk.rearrange("(s c p) q -> p s c q", s=2, p=128)
            # the very first hi blocks go in ktile-halves: class-A cp0/cp1
            # DRs start after ~0.5MB instead of 1MB
            nc.sync.dma_start(wq_s[:, 0, 0:4, :], wq_src[:, 0, 0:4, :])
            nc.sync.dma_start(xts["q"][:, 0, 0:4, :], xq_src[:, 0, 0:4, 0:SL])
            nc.sync.dma_start(wq_s[:, 0, 4:8, :], wq_src[:, 0, 4:8, :])
            nc.sync.dma_start(xts["q"][:, 0, 4:8, :], xq_src[:, 0, 4:8, 0:SL])
            nc.sync.dma_start(wq_s[:, 1, :, :], wq_src[:, 1, :, :])
            nc.sync.dma_start(xts["q"][:, 1, :, :], xq_src[:, 1, :, 0:SL])
            emit_w_dmas("bqk")
            for s in range(2):
                nc.sync.dma_start(wk_s[:, s, :, :], wk_src[:, s, :, :])
                nc.sync.dma_start(xts["k"][:, s, :, :], xk_src[:, s, :, 0:SL])
            emit_w_dmas("first")
            # full QK(0): the later m-chunks overlap the wk/xk DMA waits
            psh0 = {}
            for m in range(4):
                emit_qk_unit(0, xts["q"], wq_s, qt_s, bqk_s[:, 0:4], m, None, psh0)
            for m in range(4):
                emit_qk_unit(0, xts["k"], wk_s, kt_s, bqk_s[:, 4:8], m, None, psh0)
            emit_w_dmas("v")
            xts["v"] = emit_x_dma(0, xv, "v")

            # ---------- attention + pipeline ----------
            ctxT8_all = {}   # n -> [128, 2, 4, 512] fp8 hi/lo tile

            # Two filler queues paced against the Act exp stream:
            #  - dq: hard-deadline entries (due, thunk), FIFO in due order —
            #    projection units gated just-in-time per m-chunk / slice
            #  - sq: soft entries (deferred C units), popped on margin only
            dq = []
            sq = []
            margin = float(_os.environ.get("K_MARGIN", 2500))

            def pop_fillers(now=None, force_all=False):
                # dq pops strictly at deadline (margin pops would run ahead
                # of the DMA stream and stall the in-order PE); sq (C units)
                # absorbs the margin slack
                while dq and (force_all
                              or (now is not None and dq[0][0] <= now)):
                    dq.pop(0)[1]()
                while sq and (force_all or st["pe"] < st["act"] + margin):
                    sq.pop(0)()

            # prologue PE work ran concurrent with the x/w DMA stream; start
            # the pacing race fresh at the item stream
            st["act"] = st["pe"]

            def gidx(n, ii):
                return n * 1000 + ii

            def queue_qk_m(n1, m, xtd, psh, n_due, ii_due):
                # spread: 2 entries per item starting at (n_due, ii_due)
                ents = []
                for ci in range(3):
                    ents.append(lambda m=m, ci=ci: emit_qk_unit(
                        n1, xtd["q"], wq_s, qt_s, bqk_s[:, 0:4], m, ci, psh))
                for ci in range(3):
                    ents.append(lambda m=m, ci=ci: emit_qk_unit(
                        n1, xtd["k"], wk_s, kt_s, bqk_s[:, 4:8], m, ci, psh))
                for i, e in enumerate(ents):
                    dq.append((gidx(n_due, ii_due + i // 2), e))

            def queue_v(n1, xtd, psh, n_due, ii_due, spread=2):
                # AV consumes the diagonal key blocks (high tt) first and
                # heads 0-3 (hf=0) before 4-7
                ents = [lambda tt=tt, hf=hf: emit_v_unit(
                            n1, xtd["v"], tt, hf, None, psh)
                        for tt in range(4) for hf in range(2)]
                for i, e in enumerate(ents):
                    dq.append((gidx(n_due, ii_due + i // spread), e))

            # V(0) just-in-time within slice 0 (after its DMA lands); wo
            # rides the queue behind the slice-1 x prefetch
            # V(0) pops at items 5-8: after the wv/xv0 DMAs land (no in-order
            # PE stall) and before the first AV drain (at item 10)
            psh0b = {}
            queue_v(0, xts, psh0b, 0, 5)
            dq.append((gidx(0, 9), lambda: emit_w_dmas("rest")))

            for n in range(NS):
                if n + 1 < NS:
                    # eager whole-slice x prefetch + JIT-gated proj units;
                    # V(n+1)+m0(n+1) spread over the tail items of slice n
                    n1 = n + 1
                    nxts = {"q": emit_x_dma(n1, xq, "q"),
                            "k": emit_x_dma(n1, xk, "k"),
                            "v": emit_x_dma(n1, xv, "v")}
                    pshn = {}
                    ni = HD * 2 * (n + 1)    # items in slice n
                    npg = 2 * (n1 + 1)       # items per head in slice n1
                    queue_v(n1, nxts, pshn, n, ni - 7)
                    queue_qk_m(n1, 0, nxts, pshn, n, ni - 3)
                    for j in (1, 2, 3):
                        queue_qk_m(n1, j, nxts, pshn,
                                   n1, max(0, 2 * j * npg - 7))

                nkb = 4 * n + 4
                ngrp = nkb // 2
                ctxT8_n = ctxT8p.tile([128, 2, 4, 512], F8, tag="ctxT8",
                                      name=f"ctxT8_{n}")
                ctxT8_all[n] = ctxT8_n
                psT_cur = [None]

                def emit_s_exp(h, g, pts):
                    # Both banks of a group share the group's column base so a
                    # single 2-bank exp covers them (the extra computed scores
                    # land in q-chunks the AV stage never reads).
                    mc, po = h // 2, (h % 2) * 64
                    sps = psS.tile([128, 2, 512], F32, tag="s",
                                   name=f"s{n}_{h}_{g}")
                    pt = ptp.tile([128, 2, 512], BF16, tag="pt",
                                  name=f"pt{n}_{h}_{g}")
                    c0a = max(0, 2 * g * 128 - n * SL)
                    for i in range(2):
                        kb = 2 * g + i
                        # per-kb trim: queries before the key block are dead
                        # (AV skips them); exp still reads from c0a, the
                        # stale-psum cols it covers land in dead pt slots.
                        # f32r needs >=256 moving cols for 1 cyc/row.
                        c0i = min(max(0, kb * 128 - n * SL), SL - 256)
                        mm(sps[:, i, c0i:],
                           kt_s[po:po + 64, mc, kb * 128:(kb + 1) * 128],
                           qt_s[po:po + 64, mc, n * SL + c0i:(n + 1) * SL],
                           start=True, stop=True)
                    if dbg and n == 0 and g == 0:
                        sd = outsp.tile([128, 2, 512], F32, tag="sd", bufs=1,
                                        name=f"sd{n}_{h}_{g}")
                        nc.vector.tensor_copy(sd[:, :, :], sps[:, :, :])
                        nc.sync.dma_start(
                            s_dbg[:, h * 1024:(h + 1) * 1024], sd[:, :, :])
                    st["act"] += (2 * (512 - c0a)) * ACT_NS + 185.0
                    nc.scalar.activation(
                        pt[:, :, c0a:], sps[:, :, c0a:],
                        func=mybir.ActivationFunctionType.Exp,
                        scale=EXP_SCALE)
                    if dbg and n == 0 and g == 0:
                        pd = outsp.tile([128, 2, 512], F32, tag="pd", bufs=1,
                                        name=f"pd{n}_{h}_{g}")
                        nc.vector.tensor_copy(pd[:, :, :], pt[:, :, :])
                        nc.sync.dma_start(
                            pt_dbg[:, h * 1024:(h + 1) * 1024], pd[:, :, :])
                    pts[(h, g)] = pt

                def emit_av(h, g, avp, pts, first_grp, last_grp):
                    # PSUM start=True lazily zero-marks the WHOLE bank, so
                    # only the first emitted write into the bank may use it;
                    # later first-writes per region overwrite via the
                    # pending-zero flags.  Accumulation order over kb is free.
                    pt = pts.pop((h, g))
                    for i in range(2):
                        kb = 2 * g + i
                        if kb >= 4 * n:  # diagonal block: causal mask
                            col0 = max(0, kb * 128 - n * SL)
                            (nc.vector if mask_eng_env
                             else nc.gpsimd).tensor_mul(
                                pt[:, i, col0:col0 + 128],
                                pt[:, i, col0:col0 + 128], mi_s[:, 0:128])
                    started = [not (g == first_grp)]
                    for i in range(2):
                        kb = 2 * g + i
                        for qc in range(4):
                            if kb > 4 * n + qc:
                                continue
                            last = (g == last_grp) and (
                                kb == min(1, 4 * n + qc))
                            mm(avp[:, qc, 0:65],
                               pt[:, i, qc * 128:(qc + 1) * 128],
                               vh_s[:, kb, h // 4, (h % 4) * 65:(h % 4) * 65 + 65],
                               start=not started[0],
                               stop=last,
                               skip_group_check=True)
                            started[0] = True

                def emit_tail(h, avp):
                    # normalize: ctx_t[q, d] = 4 * av[q, d] / (av[q, 64]/8)
                    mc, po = h // 2, (h % 2) * 64
                    rcp = smallp.tile([128, 4, 1], F32, tag="rcp",
                                      name=f"rcp{n}_{h}")
                    nc.vector.reciprocal(rcp[:, :], avp[:, :, 64:65])
                    ctx_t = ctxp.tile([128, 4, DK], BF16, tag="ctx",
                                      name=f"ctx{n}_{h}")
                    if dbg and n == 0:
                        avd = outsp.tile([128, 4, 65], F32, tag="avd", bufs=1,
                                         name=f"avd{n}_{h}")
                        nc.vector.tensor_copy(avd[:, :, :], avp[:, :, :65])
                        nc.sync.dma_start(
                            av_dbg[:, h * 260:(h + 1) * 260], avd[:, :, :])
                    for qc in range(4):
                        nc.vector.tensor_scalar_mul(
                            ctx_t[:, qc, :], avp[:, qc, 0:DK], rcp[:, qc, :])
                    if dbg and n == 0:
                        ctd = outsp.tile([128, 4, DK], F32, tag="ctd", bufs=1,
                                         name=f"ctd{n}_{h}")
                        nc.vector.tensor_copy(ctd[:, :, :], ctx_t[:, :, :])
                        nc.sync.dma_start(
                            ct_dbg[:, h * 256:(h + 1) * 256], ctd[:, :, :])
                    # transpose to dims-major; 2 heads share one psum tile
                    if h % 2 == 0:
                        psT_cur[0] = psG.tile([128, 512], BF16, tag="t",
                                              name=f"t{n}_{mc}")
                    psT = psT_cur[0]
                    for qc in range(4):
                        st["pe"] += 128 * PE_NS
                        nc.tensor.transpose(
                            psT[po:po + 64, qc * 128:(qc + 1) * 128],
                            ctx_t[:, qc, :], mi_s[:, 128:256])
                    if h % 2 == 1:
                        # fp8 hi/lo split of the transposed ctx chunk
                        nc.vector.tensor_copy(ctxT8_n[:, 0, mc, :], psT[:, :])
                        nc.vector.tensor_sub(ctxT8_n[:, 1, mc, :], psT[:, :],
                                             ctxT8_n[:, 0, mc, :])

                # flat (head, group) pipeline: AV lags S/exp by one item so
                # head boundaries don't bunch the Act queue against psS WARs
                gorder = list(range(ngrp - 1, -1, -1))  # diag groups first
                items = [(h, g) for h in range(HD) for g in gorder]
                pts = {}
                avps = {}
                pending = []

                def drain_av():
                    ph, pg = pending.pop(0)
                    emit_av(ph, pg, avps[ph], pts, gorder[0], gorder[-1])
                    if pg == gorder[-1]:
                        emit_tail(ph, avps.pop(ph))

                for ii, (h, g) in enumerate(items):
                    if g == gorder[0]:
                        avps[h] = psAV.tile([128, 4, 128], F32, tag="av",
                                            name=f"av{n}_{h}")
                    emit_s_exp(h, g, pts)
                    pending.append((h, g))
                    # deep lag lets slice-0's V wait out its DMA before the
                    # first AV; the last slice drains shallow so its tails
                    # (and so the C(3) units) finish earlier — there the S
                    # stream is already act-paced, so AV never waits on pt
                    if len(pending) > (9 if n < NS - 1 else 2):
                        drain_av()
                    pop_fillers(now=gidx(n, ii))
                while pending:
                    drain_av()

                # C units for this slice become available now
                sq.extend(c_units(n, ctxT8_n))
                if dbg:
                    for c in range(4):
                        ct = outsp.tile([128, 512], F32, tag="dbg", bufs=1,
                                        name=f"cdb{n}_{c}")
                        h8 = outsp.tile([128, 512], F32, tag="dbg8", bufs=1,
                                        name=f"cdb8{n}_{c}")
                        nc.vector.tensor_copy(h8[:, :], ctxT8_n[:, 0, c, :])
                        nc.vector.tensor_add(ct[:, :], h8[:, :],
                                             ctxT8_n[:, 1, c, :])
                        nc.sync.dma_start(
                            ctx_dbg[:, (n * 4 + c) * 512:(n * 4 + c + 1) * 512],
                            ct[:, :])

            # ---------- drain remaining fillers (incl. all deferred C) ----
            pop_fillers(force_all=True)

            if dbg:
                nc.sync.dma_start(qt_dbg[:, :], qt_s[:, :, :].bitcast(F32))
                nc.sync.dma_start(kt_dbg[:, :], kt_s[:, :, :].bitcast(F32))
                for kb in range(NKB):
                    for hf in range(2):
                        vt = outsp.tile([128, 512], F32, tag="dbg", bufs=1,
                                        name=f"vdb{kb}_{hf}")
                        nc.vector.tensor_copy(vt[:, 0:260], vh_s[:, kb, hf, :])
                        nc.sync.dma_start(
                            vh_dbg[:, kb * 520 + hf * 260:
                                   kb * 520 + (hf + 1) * 260], vt[:, 0:260])

    nc.compile()
    return nc


_NC = None
LAST_RESULTS = None


def _hl(x, f8):
    h = x.astype(f8)
    l = (x - h.astype(np.float32)).astype(f8)
    return np.concatenate([h, l], axis=0)


def kernel(**inputs):
    global _NC, LAST_RESULTS
    import os
    import ml_dtypes
    if _NC is None:
        _NC = _build_nc()

    f8 = ml_dtypes.float8_e4m3
    bf = ml_dtypes.bfloat16
    f = lambda a: np.asarray(a, dtype=np.float32)
    q, k, v = f(inputs["q"]), f(inputs["k"]), f(inputs["v"])
    wq_w, wq_b = f(inputs["wq_w"]), f(inputs["wq_b"])
    wk_w, wk_b = f(inputs["wk_w"]), f(inputs["wk_b"])
    wv_w, wv_b = f(inputs["wv_w"]), f(inputs["wv_b"])
    wo_w, wo_b = f(inputs["wo_w"]), f(inputs["wo_b"])

    msk = np.ascontiguousarray(
        (np.arange(128)[None, :] >= np.arange(128)[:, None])).astype(bf)
    idn = np.eye(128).astype(bf)
    mi = np.ascontiguousarray(np.concatenate([msk, idn], axis=1))

    gmaps = []
    for g in range(2):
        sl = slice(g * GW, (g + 1) * GW)
        wq8 = _hl(np.ascontiguousarray(wq_w[sl].T * SQ), f8)
        wk8 = _hl(np.ascontiguousarray(wk_w[sl].T * SQ), f8)
        wvT = np.zeros((D, AUGW), np.float32)
        vbias = np.zeros((AUGW,), np.float32)
        for h in range(HD):
            wvT[:, h * 65:h * 65 + 64] = wv_w[g * GW + h * 64:
                                              g * GW + (h + 1) * 64].T * SV
            vbias[h * 65:h * 65 + 64] = wv_b[g * GW + h * 64:
                                             g * GW + (h + 1) * 64] * SV
            vbias[h * 65 + 64] = SONE
        wv8 = _hl(wvT, f8)
        wo8 = _hl(np.ascontiguousarray(wo_w[:, sl].T * SO), f8)
        bqT = np.ascontiguousarray((wq_b[sl] * SQ).reshape(4, 128).T)
        bkT = np.ascontiguousarray((wk_b[sl] * SQ).reshape(4, 128).T)
        gmaps.append(dict(wq=wq8, wk=wk8, wv=wv8, wo=wo8,
                          bqk=np.ascontiguousarray(
                              np.concatenate([bqT, bkT], axis=1)),
                          vb=vbias, mi=mi))

    bmaps = []
    for b in range(B):
        bmaps.append(dict(
            xq=_hl(np.ascontiguousarray(q[b].T), f8),
            xk=_hl(np.ascontiguousarray(k[b].T), f8),
            xv=_hl(np.ascontiguousarray(v[b].T), f8)))

    in_maps = [dict(**bmaps[c // 2], **gmaps[c % 2]) for c in range(8)]

    trace = bool(int(os.environ.get("KERNEL_TRACE", "0")))
    res = run_bass_kernel_spmd(_NC, in_maps, list(range(8)), trace=trace)
    LAST_RESULTS = res

    out = np.empty((B, L, D), np.float32)
    for b in range(B):
        out[b] = (np.asarray(res.results[2 * b]["outp"], np.float32)
                  + np.asarray(res.results[2 * b + 1]["outp"], np.float32)
                  + wo_b[None, :])
    return out


# revision 59
# speedup vs baseline: 1.0077x; 1.0008x over previous
"""Causal MHA (B=4, L=2048, D=1024, H=16) on 8 NeuronCores — fused pipeline.

Sharding: core c -> (batch b = c//2, head-group g = c%2), 8 heads/core.
wq/wk/wv column-parallel, wo row-parallel; host sums the two half-group
partials per batch and adds wo_b.

Single dataflow pipeline per core.  All four projection-class matmuls
(Q/K/V proj and the output projection C) run in fp8e4 DoubleRow perf mode
(0.5 PE cycles/row, contraction 2x128 per instr).  Precision is held at
~bf16 level with a 3-slot hi/lo decomposition: operand a = a_hi + a_lo
(both e4m3, host-split), product = ah*bh + al*bh + ah*bl (lo*lo dropped),
so each 8-ktile contraction costs 12 DR instrs = 6N cycles vs bf16's 8N.
Operands are pre-scaled by powers of 2 into e4m3's normal range; the
rescales fold into the exp scale (S arrives as 1024*S, exp applies
scale=1/1024), the vh ones-column (denominator carries the V scale so
ctx = num*rcp lands at 4*ctx, e4m3-ranged), and the C eviction (1/64).

Attention core: S = K^T@Q in f32r (1 cyc/row, 64-wide contraction), exp
on Act -> pt bf16, AV transposed (psum[q, 4, 65] += pt_chunk.T @ vh_kb)
in bf16, diag mask-mul on gpsimd (Pool is otherwise idle; DVE is loaded
with evictions).  The S/AV path cannot ride DoubleRow: fp8 quantization
of Q/K/P injects ~3-5% attention-weight noise, over the 2e-2 gate.

ctxT is produced as an fp8 hi/lo pair (DVE quantize + subtract off the
transpose psum) feeding the DR output projection.  Scheduling: flat item
stream with two filler queues — dq holds proj units with just-in-time
deadlines (per m-chunk: qt/kt chunk j lands right before head pair 2j;
V(n) before slice n's first AV drain, which lags 10 items so slice 0's V
can wait out its DMA), sq holds deferred C units popped when the Act
stream is ahead.  dq pops on deadline only: margin-popping would run
ahead of the serialized DMA stream and stall the in-order PE.  x tiles
are whole-slice single-DMA and prefetched a slice ahead; weights are one
DMA each (HWDGE costs ~665ns/DMA, so descriptor count matters).  Last-
slice C psums alternate into the freed S banks and their evictions
alternate Act/DVE to double the tail pipeline.  PSUM: 2x S sets (4
banks) + AV (1) + general (2) + transpose (1) = 8 banks.
TimelineSim: 211,339 ns (prior bf16 kernel: 245,461; rel err 4.6e-3).
"""

import numpy as np
import os as _os

import concourse.bacc as bacc
import concourse.bass as bass
import concourse.mybir as mybir
import concourse.tile as tile
from concourse.bass_utils import run_bass_kernel_spmd

F32 = mybir.dt.float32
F32R = mybir.dt.float32r
BF16 = mybir.dt.bfloat16
F8 = mybir.dt.float8e4
DR = mybir.MatmulPerfMode.DoubleRow

B, L, D, H, DK = 4, 2048, 1024, 16, 64
HD = 8              # heads per core
GW = 512            # head-group width
AUGW = HD * (DK + 1)  # 520
NCH = D // 128      # 8 contraction chunks
SL = 512            # token slice
NS = L // SL        # 4
NKB = L // 128      # 16

PE_NS = 1.0 / 2.4   # ns per PE cycle at full clock
ACT_NS = 1.0 / 1.2  # ns per Act cycle

SQ = 32.0 / (8.0 ** 0.5)   # scale folded into wq/wk before e4m3 split
SV = 32.0                  # scale folded into wv
SONE = 8.0                 # vh ones-column value -> ctx lands at 4*ctx
SO = 16.0                  # scale folded into wo
C_EVICT = 1.0 / 64.0       # (4*ctx)*(16*wo) -> /64
EXP_SCALE = 1.0 / 1024.0   # qt*kt = 1024*S_true


def _build_nc(dbg=False):
    nc = bacc.Bacc("TRN2", target_bir_lowering=False, debug=False, num_devices=8)

    # x/w tensors carry [hi; lo] e4m3 blocks stacked on the row (contraction)
    # axis; host does the split.
    xq = nc.dram_tensor("xq", [2 * D, L], F8, kind="ExternalInput").ap()
    xk = nc.dram_tensor("xk", [2 * D, L], F8, kind="ExternalInput").ap()
    xv = nc.dram_tensor("xv", [2 * D, L], F8, kind="ExternalInput").ap()
    wq = nc.dram_tensor("wq", [2 * D, GW], F8, kind="ExternalInput").ap()
    wk = nc.dram_tensor("wk", [2 * D, GW], F8, kind="ExternalInput").ap()
    wv = nc.dram_tensor("wv", [2 * D, AUGW], F8, kind="ExternalInput").ap()
    wo = nc.dram_tensor("wo", [2 * GW, D], F8, kind="ExternalInput").ap()
    bqk = nc.dram_tensor("bqk", [128, 8], F32, kind="ExternalInput").ap()
    vb = nc.dram_tensor("vb", [AUGW], F32, kind="ExternalInput").ap()
    mi = nc.dram_tensor("mi", [128, 256], BF16, kind="ExternalInput").ap()
    outp = nc.dram_tensor("outp", [L, D], BF16, kind="ExternalOutput").ap()
    if dbg:
        qt_dbg = nc.dram_tensor("qt_dbg", [128, 4 * L], F32, kind="ExternalOutput").ap()
        kt_dbg = nc.dram_tensor("kt_dbg", [128, 4 * L], F32, kind="ExternalOutput").ap()
        vh_dbg = nc.dram_tensor("vh_dbg", [128, NKB * 520], F32,
                                kind="ExternalOutput").ap()
        ctx_dbg = nc.dram_tensor("ctx_dbg", [128, 4 * L], F32,
                                 kind="ExternalOutput").ap()
        av_dbg = nc.dram_tensor("av_dbg", [128, HD * 4 * 65], F32,
                                kind="ExternalOutput").ap()
        ct_dbg = nc.dram_tensor("ct_dbg", [128, HD * 4 * DK], F32,
                                kind="ExternalOutput").ap()
        s_dbg = nc.dram_tensor("s_dbg", [128, HD * 1024], F32,
                               kind="ExternalOutput").ap()
        pt_dbg = nc.dram_tensor("pt_dbg", [128, HD * 1024], F32,
                                kind="ExternalOutput").ap()

    mask_eng_env = _os.environ.get("K_MASKDVE")

    with tile.TileContext(nc) as tc:
        with (
            tc.tile_pool(name="persist", bufs=1) as persist,
            tc.tile_pool(name="xin", bufs=4 if dbg else 6) as xinp,
            tc.tile_pool(name="pt", bufs=10) as ptp,
            tc.tile_pool(name="ctx", bufs=4) as ctxp,
            tc.tile_pool(name="ctxT8", bufs=4) as ctxT8p,
            tc.tile_pool(name="small", bufs=8) as smallp,
            tc.tile_pool(name="outs", bufs=4) as outsp,
            tc.tile_pool(name="psS", bufs=2, space="PSUM") as psS,
            tc.tile_pool(name="psAV", bufs=1, space="PSUM") as psAV,
            tc.tile_pool(name="psG", bufs=1, space="PSUM") as psG,
        ):
            # ---- persistent SBUF ----
            # weights: [128, s(hi/lo), ktile, cols]
            wq_s = persist.tile([128, 2, NCH, GW], F8, tag="wq")
            wk_s = persist.tile([128, 2, NCH, GW], F8, tag="wk")
            wv_s = persist.tile([128, 2, NCH, AUGW], F8, tag="wv")
            wo_s = persist.tile([128, 2, 4, D], F8, tag="wo")
            qt_s = persist.tile([128, 4, L], F32R, tag="qt")
            kt_s = persist.tile([128, 4, L], F32R, tag="kt")
            vh_s = persist.tile([128, NKB, 2, 260], BF16, tag="vh")
            bqk_s = persist.tile([128, 8], F32, tag="bqk")
            vb_s = persist.tile([128, AUGW], F32, tag="vb")
            mi_s = persist.tile([128, 256], BF16, tag="mi")

            def emit_w_dmas(which, split=False):
                if which == "bqk":
                    nc.sync.dma_start(bqk_s[:, :], bqk[:, :])
                elif which == "first":
                    nc.sync.dma_start(mi_s[:, :], mi[:, :])
                    vb_bcast = bass.AP(tensor=vb.tensor, offset=vb.offset,
                                       ap=[[0, 128], [1, AUGW]])
                    nc.sync.dma_start(vb_s[:, :], vb_bcast)
                elif which in ("q", "k", "v"):
                    w_s, w_d = {"q": (wq_s, wq), "k": (wk_s, wk),
                                "v": (wv_s, wv)}[which]
                    src = w_d.rearrange("(s c p) q -> p s c q", s=2, p=128)
                    if split:
                        for s in range(2):
                            nc.sync.dma_start(w_s[:, s, :, :], src[:, s, :, :])
                    else:
                        nc.sync.dma_start(w_s[:, :, :, :], src)
                else:
                    nc.sync.dma_start(
                        wo_s[:, :, :, :],
                        wo.rearrange("(s c p) q -> p s c q", s=2, p=128))

            # ---------- pacing counters (ns, at full clocks) ----------
            st = {"pe": 0.0, "act": 0.0}

            def mm(*args, **kw):
                out = args[0]
                st["pe"] += out.free_size() * PE_NS
                nc.tensor.matmul(*args, **kw)

            def mmdr(*args, **kw):
                out = args[0]
                st["pe"] += out.free_size() * 0.5 * PE_NS
                nc.tensor.matmul(*args, perf_mode=DR, **kw)

            # ---------- projection / output-projection units ----------
            def emit_x_dma(n, src, tag, split=False):
                # whole-slice tile [128, s(hi/lo), 8 ktiles, SL], one DMA
                # (two when split: hi first so class-A matmuls start earlier)
                t = xinp.tile([128, 2, NCH, SL], F8, tag="x", name=f"x{tag}{n}")
                s_ap = src.rearrange("(s c p) q -> p s c q", s=2, p=128)[
                    :, :, :, n * SL:(n + 1) * SL]
                if split:
                    for s in range(2):
                        nc.sync.dma_start(t[:, s, :, :], s_ap[:, s, :, :])
                else:
                    nc.sync.dma_start(t[:, :, :, :], s_ap)
                return t

            # 3-slot fp8 classes: (ws, xs) in A=(hi,hi), B=(lo,hi), C=(hi,lo)
            SLOT3 = ((0, 0), (1, 0), (0, 1))

            def emit_qk_unit(n, xt, w_s, dst, b_s, m, ci, psh):
                # ci: slot-class index 0..2 (emission granularity), or None
                cis = range(3) if ci is None else (ci,)
                if cis[0] == 0:
                    psh[m] = psG.tile([128, 512], F32, tag="g", bufs=2,
                                      name=f"qk{n}_{m}")
                ps = psh[m]
                for c3 in cis:
                    ws, xs = SLOT3[c3]
                    for cp in range(4):
                        cg = cp * 2
                        mmdr(ps[:, :],
                             w_s[:, ws, cg:cg + 2, m * 128:(m + 1) * 128],
                             xt[:, xs, cg:cg + 2, :],
                             start=(c3 == 0 and cp == 0),
                             stop=(c3 == 2 and cp == 3))
                if cis[-1] == 2:
                    del psh[m]
                    nc.vector.tensor_scalar_add(
                        dst[:, m, n * SL:(n + 1) * SL], ps[:, :],
                        b_s[:, m:m + 1])

            def emit_v_unit(n, xt, tt, hf, ci, psh):
                cis = range(3) if ci is None else (ci,)
                if cis[0] == 0:
                    psh[(tt, hf)] = psG.tile([128, 512], F32, tag="g", bufs=2,
                                             name=f"v{n}_{tt}_{hf}")
                ps = psh[(tt, hf)]
                for c3 in cis:
                    xs, ws = SLOT3[c3]  # lhsT is x here
                    for cp in range(4):
                        cg = cp * 2
                        mmdr(ps[:, 0:260],
                             xt[:, xs, cg:cg + 2, tt * 128:(tt + 1) * 128],
                             wv_s[:, ws, cg:cg + 2,
                                  hf * 260:(hf + 1) * 260],
                             start=(c3 == 0 and cp == 0),
                             stop=(c3 == 2 and cp == 3))
                if cis[-1] == 2:
                    del psh[(tt, hf)]
                    kb = n * 4 + tt
                    nc.vector.tensor_add(
                        vh_s[:, kb, hf, :],
                        ps[:, 0:260], vb_s[:, hf * 260:(hf + 1) * 260])

            def emit_c_unit(n, tt, n2, ctxT8_n):
                if n >= NS - 1 and (tt + n2) % 2 == 0:
                    # the S psum banks are free once the last exps retire;
                    # alternating pools doubles the tail eviction pipeline
                    ps2 = psS.tile([128, 2, 512], F32, tag="s",
                                   name=f"c{n}_{tt}_{n2}")
                    ps = ps2[:, 0, :]
                else:
                    ps = psG.tile([128, 512], F32, tag="g", bufs=2,
                                  name=f"c{n}_{tt}_{n2}")
                # cp0 slots first (ctx chunks 0-1 are ready before 2-3 at the
                # tail), ctx-lo x cp1 last — gives the final head's hi/lo
                # split maximal runway; psum accumulation order is free
                order = ((0, 0), (1, 0), (2, 0), (0, 1), (2, 1), (1, 1))
                for i, (c3, cp) in enumerate(order):
                    s_ctx, s_wo = SLOT3[c3]
                    mmdr(ps[:, :],
                         ctxT8_n[:, s_ctx, cp * 2:cp * 2 + 2,
                                 tt * 128:(tt + 1) * 128],
                         wo_s[:, s_wo, cp * 2:cp * 2 + 2,
                              n2 * 512:(n2 + 1) * 512],
                         start=(i == 0), stop=(i == 5))
                ot = outsp.tile([128, 512], BF16, tag="ot", name=f"ot{n}_{tt}_{n2}")
                if n >= NS - 1:
                    # Act is done with exps by the time these pop; DVE still
                    # has the last ctxT8 hi/lo work queued
                    nc.scalar.activation(ot[:, :], ps[:, :],
                                         func=mybir.ActivationFunctionType.Copy,
                                         scale=C_EVICT)
                else:
                    nc.vector.tensor_scalar_mul(ot[:, :], ps[:, :], C_EVICT)
                nc.sync.dma_start(
                    outp[(n * 4 + tt) * 128:(n * 4 + tt + 1) * 128,
                         n2 * 512:(n2 + 1) * 512], ot[:, :])

            def c_units(n, ctxT8_n):
                units = []
                for tt in range(4):
                    for n2 in range(2):
                        units.append(lambda n=n, tt=tt, n2=n2: emit_c_unit(
                            n, tt, n2, ctxT8_n))
                return units

            # ---------- prologue: DMAs + Q/K m0 only (act starts ASAP) ----
            # interleaved hi-first DMA order so class-A DR matmuls can start
            # after ~1MB of transfer; the small DMAs queue behind the first
            # consumer-critical MBs (each dma_start costs ~665ns of HWDGE):
            # bqk before the first qt evict, mi/vb before the first mask/V
            # evict (not needed until drain item 10 / V pops)
            wq_src = wq.rearrange("(s c p) q -> p s c q", s=2, p=128)
            wk_src = wk.rearrange("(s c p) q -> p s c q", s=2, p=128)
            xts = {}
            xts["q"] = xinp.tile([128, 2, NCH, SL], F8, tag="x", name="xq0")
            xts["k"] = xinp.tile([128, 2, NCH, SL], F8, tag="x", name="xk0")
            xq_src = xq.rearrange("(s c p) q -> p s c q", s=2, p=128)
            xk_src = xk.rearrange("(s c p) q -> p s c q", s=2, p=128)
            # the very first hi blocks go in ktile-halves: class-A cp0/cp1
            # DRs start after ~0.5MB instead of 1MB
            nc.sync.dma_start(wq_s[:, 0, 0:4, :], wq_src[:, 0, 0:4, :])
            nc.sync.dma_start(xts["q"][:, 0, 0:4, :], xq_src[:, 0, 0:4, 0:SL])
            nc.sync.dma_start(wq_s[:, 0, 4:8, :], wq_src[:, 0, 4:8, :])
            nc.sync.dma_start(xts["q"][:, 0, 4:8, :], xq_src[:, 0, 4:8, 0:SL])
            nc.sync.dma_start(wq_s[:, 1, :, :], wq_src[:, 1, :, :])
            nc.sync.dma_start(xts["q"][:, 1, :, :], xq_src[:, 1, :, 0:SL])
            emit_w_dmas("bqk")
            for s in range(2):
                nc.sync.dma_start(wk_s[:, s, :, :], wk_src[:, s, :, :])
                nc.sync.dma_start(xts["k"][:, s, :, :], xk_src[:, s, :, 0:SL])
            emit_w_dmas("first")
            # full QK(0): the later m-chunks overlap the wk/xk DMA waits
            psh0 = {}
            for m in range(4):
                emit_qk_unit(0, xts["q"], wq_s, qt_s, bqk_s[:, 0:4], m, None, psh0)
            for m in range(4):
                emit_qk_unit(0, xts["k"], wk_s, kt_s, bqk_s[:, 4:8], m, None, psh0)
            emit_w_dmas("v")
            xts["v"] = emit_x_dma(0, xv, "v")

            # ---------- attention + pipeline ----------
            ctxT8_all = {}   # n -> [128, 2, 4, 512] fp8 hi/lo tile

            # Two filler queues paced against the Act exp stream:
            #  - dq: hard-deadline entries (due, thunk), FIFO in due order —
            #    projection units gated just-in-time per m-chunk / slice
            #  - sq: soft entries (deferred C units), popped on margin only
            dq = []
            sq = []
            margin = float(_os.environ.get("K_MARGIN", 2500))

            def pop_fillers(now=None, force_all=False):
                # dq pops strictly at deadline (margin pops would run ahead
                # of the DMA stream and stall the in-order PE); sq (C units)
                # absorbs the margin slack
                while dq and (force_all
                              or (now is not None and dq[0][0] <= now)):
                    dq.pop(0)[1]()
                while sq and (force_all or st["pe"] < st["act"] + margin):
                    sq.pop(0)()

            # prologue PE work ran concurrent with the x/w DMA stream; start
            # the pacing race fresh at the item stream
            st["act"] = st["pe"]

            def gidx(n, ii):
                return n * 1000 + ii

            def queue_qk_m(n1, m, xtd, psh, n_due, ii_due):
                # spread: 2 entries per item starting at (n_due, ii_due)
                ents = []
                for ci in range(3):
                    ents.append(lambda m=m, ci=ci: emit_qk_unit(
                        n1, xtd["q"], wq_s, qt_s, bqk_s[:, 0:4], m, ci, psh))
                for ci in range(3):
                    ents.append(lambda m=m, ci=ci: emit_qk_unit(
                        n1, xtd["k"], wk_s, kt_s, bqk_s[:, 4:8], m, ci, psh))
                for i, e in enumerate(ents):
                    dq.append((gidx(n_due, ii_due + i // 2), e))

            def queue_v(n1, xtd, psh, n_due, ii_due, spread=2):
                # AV consumes the diagonal key blocks (high tt) first and
                # heads 0-3 (hf=0) before 4-7
                ents = [lambda tt=tt, hf=hf: emit_v_unit(
                            n1, xtd["v"], tt, hf, None, psh)
                        for tt in range(4) for hf in range(2)]
                for i, e in enumerate(ents):
                    dq.append((gidx(n_due, ii_due + i // spread), e))

            # V(0) just-in-time within slice 0 (after its DMA lands); wo
            # rides the queue behind the slice-1 x prefetch
            # V(0) pops at items 5-8: after the wv/xv0 DMAs land (no in-order
            # PE stall) and before the first AV drain (at item 10)
            psh0b = {}
            queue_v(0, xts, psh0b, 0, 5)
            dq.append((gidx(0, 9), lambda: emit_w_dmas("rest")))

            for n in range(NS):
                if n + 1 < NS:
                    # eager whole-slice x prefetch + JIT-gated proj units;
                    # V(n+1)+m0(n+1) spread over the tail items of slice n
                    n1 = n + 1
                    nxts = {"q": emit_x_dma(n1, xq, "q"),
                            "k": emit_x_dma(n1, xk, "k"),
                            "v": emit_x_dma(n1, xv, "v")}
                    pshn = {}
                    ni = HD * 2 * (n + 1)    # items in slice n
                    npg = 2 * (n1 + 1)       # items per head in slice n1
                    queue_v(n1, nxts, pshn, n, ni - 7)
                    queue_qk_m(n1, 0, nxts, pshn, n, ni - 3)
                    for j in (1, 2, 3):
                        queue_qk_m(n1, j, nxts, pshn,
                                   n1, max(0, 2 * j * npg - 7))

                nkb = 4 * n + 4
                ngrp = nkb // 2
                ctxT8_n = ctxT8p.tile([128, 2, 4, 512], F8, tag="ctxT8",
                                      name=f"ctxT8_{n}")
                ctxT8_all[n] = ctxT8_n
                psT_cur = [None]

                def emit_s_exp(h, g, pts):
                    # Both banks of a group share the group's column base so a
                    # single 2-bank exp covers them (the extra computed scores
                    # land in q-chunks the AV stage never reads).
                    mc, po = h // 2, (h % 2) * 64
                    sps = psS.tile([128, 2, 512], F32, tag="s",
                                   name=f"s{n}_{h}_{g}")
                    pt = ptp.tile([128, 2, 512], BF16, tag="pt",
                                  name=f"pt{n}_{h}_{g}")
                    c0a = max(0, 2 * g * 128 - n * SL)
                    for i in range(2):
                        kb = 2 * g + i
                        # per-kb trim: queries before the key block are dead
                        # (AV skips them); exp still reads from c0a, the
                        # stale-psum cols it covers land in dead pt slots.
                        # f32r needs >=256 moving cols for 1 cyc/row.
                        c0i = min(max(0, kb * 128 - n * SL), SL - 256)
                        mm(sps[:, i, c0i:],
                           kt_s[po:po + 64, mc, kb * 128:(kb + 1) * 128],
                           qt_s[po:po + 64, mc, n * SL + c0i:(n + 1) * SL],
                           start=True, stop=True)
                    if dbg and n == 0 and g == 0:
                        sd = outsp.tile([128, 2, 512], F32, tag="sd", bufs=1,
                                        name=f"sd{n}_{h}_{g}")
                        nc.vector.tensor_copy(sd[:, :, :], sps[:, :, :])
                        nc.sync.dma_start(
                            s_dbg[:, h * 1024:(h + 1) * 1024], sd[:, :, :])
                    st["act"] += (2 * (512 - c0a)) * ACT_NS + 185.0
                    nc.scalar.activation(
                        pt[:, :, c0a:], sps[:, :, c0a:],
                        func=mybir.ActivationFunctionType.Exp,
                        scale=EXP_SCALE)
                    if dbg and n == 0 and g == 0:
                        pd = outsp.tile([128, 2, 512], F32, tag="pd", bufs=1,
                                        name=f"pd{n}_{h}_{g}")
                        nc.vector.tensor_copy(pd[:, :, :], pt[:, :, :])
                        nc.sync.dma_start(
                            pt_dbg[:, h * 1024:(h + 1) * 1024], pd[:, :, :])
                    pts[(h, g)] = pt

                def emit_av(h, g, avp, pts, first_grp, last_grp):
                    # PSUM start=True lazily zero-marks the WHOLE bank, so
                    # only the first emitted write into the bank may use it;
                    # later first-writes per region overwrite via the
                    # pending-zero flags.  Accumulation order over kb is free.
                    pt = pts.pop((h, g))
                    for i in range(2):
                        kb = 2 * g + i
                        if kb >= 4 * n:  # diagonal block: causal mask
                            col0 = max(0, kb * 128 - n * SL)
                            (nc.vector if mask_eng_env
                             else nc.gpsimd).tensor_mul(
                                pt[:, i, col0:col0 + 128],
                                pt[:, i, col0:col0 + 128], mi_s[:, 0:128])
                    started = [not (g == first_grp)]
                    for i in range(2):
                        kb = 2 * g + i
                        for qc in range(4):
                            if kb > 4 * n + qc:
                                continue
                            last = (g == last_grp) and (
                                kb == min(1, 4 * n + qc))
                            mm(avp[:, qc, 0:65],
                               pt[:, i, qc * 128:(qc + 1) * 128],
                               vh_s[:, kb, h // 4, (h % 4) * 65:(h % 4) * 65 + 65],
                               start=not started[0],
                               stop=last,
                               skip_group_check=True)
                            started[0] = True

                def emit_tail(h, avp):
                    # normalize: ctx_t[q, d] = 4 * av[q, d] / (av[q, 64]/8)
                    mc, po = h // 2, (h % 2) * 64
                    rcp = smallp.tile([128, 4, 1], F32, tag="rcp",
                                      name=f"rcp{n}_{h}")
                    nc.vector.reciprocal(rcp[:, :], avp[:, :, 64:65])
                    ctx_t = ctxp.tile([128, 4, DK], BF16, tag="ctx",
                                      name=f"ctx{n}_{h}")
                    if dbg and n == 0:
                        avd = outsp.tile([128, 4, 65], F32, tag="avd", bufs=1,
                                         name=f"avd{n}_{h}")
                        nc.vector.tensor_copy(avd[:, :, :], avp[:, :, :65])
                        nc.sync.dma_start(
                            av_dbg[:, h * 260:(h + 1) * 260], avd[:, :, :])
                    for qc in range(4):
                        nc.vector.tensor_scalar_mul(
                            ctx_t[:, qc, :], avp[:, qc, 0:DK], rcp[:, qc, :])
                    if dbg and n == 0:
                        ctd = outsp.tile([128, 4, DK], F32, tag="ctd", bufs=1,
                                         name=f"ctd{n}_{h}")
                        nc.vector.tensor_copy(ctd[:, :, :], ctx_t[:, :, :])
                        nc.sync.dma_start(
                            ct_dbg[:, h * 256:(h + 1) * 256], ctd[:, :, :])
                    # transpose to dims-major; 2 heads share one psum tile
                    if h % 2 == 0:
                        psT_cur[0] = psG.tile([128, 512], BF16, tag="t",
                                              name=f"t{n}_{mc}")
                    psT = psT_cur[0]
                    for qc in range(4):
                        st["pe"] += 128 * PE_NS
                        nc.tensor.transpose(
                            psT[po:po + 64, qc * 128:(qc + 1) * 128],
                            ctx_t[:, qc, :], mi_s[:, 128:256])
                    if h % 2 == 1:
                        # fp8 hi/lo split of the transposed ctx chunk
                        nc.vector.tensor_copy(ctxT8_n[:, 0, mc, :], psT[:, :])
                        nc.vector.tensor_sub(ctxT8_n[:, 1, mc, :], psT[:, :],
                                             ctxT8_n[:, 0, mc, :])

                # flat (head, group) pipeline: AV lags S/exp by one item so
                # head boundaries don't bunch the Act queue against psS WARs
                gorder = list(range(ngrp - 1, -1, -1))  # diag groups first
                items = [(h, g) for h in range(HD) for g in gorder]
                pts = {}
                avps = {}
                pending = []

                def drain_av():
                    ph, pg = pending.pop(0)
                    emit_av(ph, pg, avps[ph], pts, gorder[0], gorder[-1])
                    if pg == gorder[-1]:
                        emit_tail(ph, avps.pop(ph))

                for ii, (h, g) in enumerate(items):
                    if g == gorder[0]:
                        avps[h] = psAV.tile([128, 4, 128], F32, tag="av",
                                            name=f"av{n}_{h}")
                    emit_s_exp(h, g, pts)
                    pending.append((h, g))
                    # deep lag lets slice-0's V wait out its DMA before the
                    # first AV; the last slice drains shallow so its tails
                    # (and so the C(3) units) finish earlier — there the S
                    # stream is already act-paced, so AV never waits on pt
                    if len(pending) > (9 if n < NS - 1 else 2):
                        drain_av()
                    pop_fillers(now=gidx(n, ii))
                while pending:
                    drain_av()

                # C units for this slice become available now
                sq.extend(c_units(n, ctxT8_n))
                if dbg:
                    for c in range(4):
                        ct = outsp.tile([128, 512], F32, tag="dbg", bufs=1,
                                        name=f"cdb{n}_{c}")
                        h8 = outsp.tile([128, 512], F32, tag="dbg8", bufs=1,
                                        name=f"cdb8{n}_{c}")
                        nc.vector.tensor_copy(h8[:, :], ctxT8_n[:, 0, c, :])
                        nc.vector.tensor_add(ct[:, :], h8[:, :],
                                             ctxT8_n[:, 1, c, :])
                        nc.sync.dma_start(
                            ctx_dbg[:, (n * 4 + c) * 512:(n * 4 + c + 1) * 512],
                            ct[:, :])

            # ---------- drain remaining fillers (incl. all deferred C) ----
            pop_fillers(force_all=True)

            if dbg:
                nc.sync.dma_start(qt_dbg[:, :], qt_s[:, :, :].bitcast(F32))
                nc.sync.dma_start(kt_dbg[:, :], kt_s[:, :, :].bitcast(F32))
                for kb in range(NKB):
                    for hf in range(2):
                        vt = outsp.tile([128, 512], F32, tag="dbg", bufs=1,
                                        name=f"vdb{kb}_{hf}")
                        nc.vector.tensor_copy(vt[:, 0:260], vh_s[:, kb, hf, :])
                        nc.sync.dma_start(
                            vh_dbg[:, kb * 520 + hf * 260:
                                   kb * 520 + (hf + 1) * 260], vt[:, 0:260])

    nc.compile()
    return nc


_NC = None
LAST_RESULTS = None


def _hl(x, f8):
    h = x.astype(f8)
    l = (x - h.astype(np.float32)).astype(f8)
    return np.concatenate([h, l], axis=0)


def kernel(**inputs):
    global _NC, LAST_RESULTS
    import os
    import ml_dtypes
    if _NC is None:
        _NC = _build_nc()

    f8 = ml_dtypes.float8_e4m3
    bf = ml_dtypes.bfloat16
    f = lambda a: np.asarray(a, dtype=np.float32)
    q, k, v = f(inputs["q"]), f(inputs["k"]), f(inputs["v"])
    wq_w, wq_b = f(inputs["wq_w"]), f(inputs["wq_b"])
    wk_w, wk_b = f(inputs["wk_w"]), f(inputs["wk_b"])
    wv_w, wv_b = f(inputs["wv_w"]), f(inputs["wv_b"])
    wo_w, wo_b = f(inputs["wo_w"]), f(inputs["wo_b"])

    msk = np.ascontiguousarray(
        (np.arange(128)[None, :] >= np.arange(128)[:, None])).astype(bf)
    idn = np.eye(128).astype(bf)
    mi = np.ascontiguousarray(np.concatenate([msk, idn], axis=1))

    gmaps = []
    for g in range(2):
        sl = slice(g * GW, (g + 1) * GW)
        wq8 = _hl(np.ascontiguousarray(wq_w[sl].T * SQ), f8)
        wk8 = _hl(np.ascontiguousarray(wk_w[sl].T * SQ), f8)
        wvT = np.zeros((D, AUGW), np.float32)
        vbias = np.zeros((AUGW,), np.float32)
        for h in range(HD):
            wvT[:, h * 65:h * 65 + 64] = wv_w[g * GW + h * 64:
                                              g * GW + (h + 1) * 64].T * SV
            vbias[h * 65:h * 65 + 64] = wv_b[g * GW + h * 64:
                                             g * GW + (h + 1) * 64] * SV
            vbias[h * 65 + 64] = SONE
        wv8 = _hl(wvT, f8)
        wo8 = _hl(np.ascontiguousarray(wo_w[:, sl].T * SO), f8)
        bqT = np.ascontiguousarray((wq_b[sl] * SQ).reshape(4, 128).T)
        bkT = np.ascontiguousarray((wk_b[sl] * SQ).reshape(4, 128).T)
        gmaps.append(dict(wq=wq8, wk=wk8, wv=wv8, wo=wo8,
                          bqk=np.ascontiguousarray(
                              np.concatenate([bqT, bkT], axis=1)),
                          vb=vbias, mi=mi))

    bmaps = []
    for b in range(B):
        bmaps.append(dict(
            xq=_hl(np.ascontiguousarray(q[b].T), f8),
            xk=_hl(np.ascontiguousarray(k[b].T), f8),
            xv=_hl(np.ascontiguousarray(v[b].T), f8)))

    in_maps = [dict(**bmaps[c // 2], **gmaps[c % 2]) for c in range(8)]

    trace = bool(int(os.environ.get("KERNEL_TRACE", "0")))
    res = run_bass_kernel_spmd(_NC, in_maps, list(range(8)), trace=trace)
    LAST_RESULTS = res

    out = np.empty((B, L, D), np.float32)
    for b in range(B):
        out[b] = (np.asarray(res.results[2 * b]["outp"], np.float32)
                  + np.asarray(res.results[2 * b + 1]["outp"], np.float32)
                  + wo_b[None, :])
    return out


# revision 61
# speedup vs baseline: 1.0123x; 1.0046x over previous
"""Causal MHA (B=4, L=2048, D=1024, H=16) on 8 NeuronCores — fused pipeline.

Sharding: core c -> (batch b = c//2, head-group g = c%2), 8 heads/core.
wq/wk/wv column-parallel, wo row-parallel; host sums the two half-group
partials per batch and adds wo_b.

Single dataflow pipeline per core.  All four projection-class matmuls
(Q/K/V proj and the output projection C) run in fp8e4 DoubleRow perf mode
(0.5 PE cycles/row, contraction 2x128 per instr).  Precision is held at
~bf16 level with a 3-slot hi/lo decomposition: operand a = a_hi + a_lo
(both e4m3, host-split), product = ah*bh + al*bh + ah*bl (lo*lo dropped),
so each 8-ktile contraction costs 12 DR instrs = 6N cycles vs bf16's 8N.
Operands are pre-scaled by powers of 2 into e4m3's normal range; the
rescales fold into the exp scale (S arrives as 1024*S, exp applies
scale=1/1024), the vh ones-column (denominator carries the V scale so
ctx = num*rcp lands at 4*ctx, e4m3-ranged), and the C eviction (1/64).

Attention core: S = K^T@Q in f32r (1 cyc/row, 64-wide contraction), exp
on Act -> pt bf16, AV transposed (psum[q, 4, 65] += pt_chunk.T @ vh_kb)
in bf16, diag mask-mul on gpsimd (Pool is otherwise idle; DVE is loaded
with evictions).  The S/AV path cannot ride DoubleRow: fp8 quantization
of Q/K/P injects ~3-5% attention-weight noise, over the 2e-2 gate.

ctxT is produced as an fp8 hi/lo pair (DVE quantize + subtract off the
transpose psum) feeding the DR output projection.  Scheduling: flat item
stream with two filler queues — dq holds proj units with just-in-time
deadlines (per m-chunk: qt/kt chunk j lands right before head pair 2j;
V(n) before slice n's first AV drain, which lags 10 items so slice 0's V
can wait out its DMA), sq holds deferred C units popped when the Act
stream is ahead.  dq pops on deadline only: margin-popping would run
ahead of the serialized DMA stream and stall the in-order PE.  x tiles
are whole-slice single-DMA and prefetched a slice ahead; weights are one
DMA each (HWDGE costs ~665ns/DMA, so descriptor count matters).  Last-
slice C psums alternate into the freed S banks and their evictions
alternate Act/DVE to double the tail pipeline.  PSUM: 2x S sets (4
banks) + AV (1) + general (2) + transpose (1) = 8 banks.
TimelineSim: 211,170 ns (prior bf16 kernel: 245,461; rel err 4.6e-3).
"""

import numpy as np
import os as _os

import concourse.bacc as bacc
import concourse.bass as bass
import concourse.mybir as mybir
import concourse.tile as tile
from concourse.bass_utils import run_bass_kernel_spmd

F32 = mybir.dt.float32
F32R = mybir.dt.float32r
BF16 = mybir.dt.bfloat16
F8 = mybir.dt.float8e4
DR = mybir.MatmulPerfMode.DoubleRow

B, L, D, H, DK = 4, 2048, 1024, 16, 64
HD = 8              # heads per core
GW = 512            # head-group width
AUGW = HD * (DK + 1)  # 520
NCH = D // 128      # 8 contraction chunks
SL = 512            # token slice
NS = L // SL        # 4
NKB = L // 128      # 16

PE_NS = 1.0 / 2.4   # ns per PE cycle at full clock
ACT_NS = 1.0 / 1.2  # ns per Act cycle

SQ = 32.0 / (8.0 ** 0.5)   # scale folded into wq/wk before e4m3 split
SV = 32.0                  # scale folded into wv
SONE = 8.0                 # vh ones-column value -> ctx lands at 4*ctx
SO = 16.0                  # scale folded into wo
C_EVICT = 1.0 / 64.0       # (4*ctx)*(16*wo) -> /64
EXP_SCALE = 1.0 / 1024.0   # qt*kt = 1024*S_true


def _build_nc(dbg=False):
    nc = bacc.Bacc("TRN2", target_bir_lowering=False, debug=False, num_devices=8)

    # x/w tensors carry [hi; lo] e4m3 blocks stacked on the row (contraction)
    # axis; host does the split.
    xq = nc.dram_tensor("xq", [2 * D, L], F8, kind="ExternalInput").ap()
    xk = nc.dram_tensor("xk", [2 * D, L], F8, kind="ExternalInput").ap()
    xv = nc.dram_tensor("xv", [2 * D, L], F8, kind="ExternalInput").ap()
    wq = nc.dram_tensor("wq", [2 * D, GW], F8, kind="ExternalInput").ap()
    wk = nc.dram_tensor("wk", [2 * D, GW], F8, kind="ExternalInput").ap()
    wv = nc.dram_tensor("wv", [2 * D, AUGW], F8, kind="ExternalInput").ap()
    wo = nc.dram_tensor("wo", [2 * GW, D], F8, kind="ExternalInput").ap()
    bqk = nc.dram_tensor("bqk", [128, 8], F32, kind="ExternalInput").ap()
    vb = nc.dram_tensor("vb", [AUGW], F32, kind="ExternalInput").ap()
    mi = nc.dram_tensor("mi", [128, 256], BF16, kind="ExternalInput").ap()
    outp = nc.dram_tensor("outp", [L, D], BF16, kind="ExternalOutput").ap()
    if dbg:
        qt_dbg = nc.dram_tensor("qt_dbg", [128, 4 * L], F32, kind="ExternalOutput").ap()
        kt_dbg = nc.dram_tensor("kt_dbg", [128, 4 * L], F32, kind="ExternalOutput").ap()
        vh_dbg = nc.dram_tensor("vh_dbg", [128, NKB * 520], F32,
                                kind="ExternalOutput").ap()
        ctx_dbg = nc.dram_tensor("ctx_dbg", [128, 4 * L], F32,
                                 kind="ExternalOutput").ap()
        av_dbg = nc.dram_tensor("av_dbg", [128, HD * 4 * 65], F32,
                                kind="ExternalOutput").ap()
        ct_dbg = nc.dram_tensor("ct_dbg", [128, HD * 4 * DK], F32,
                                kind="ExternalOutput").ap()
        s_dbg = nc.dram_tensor("s_dbg", [128, HD * 1024], F32,
                               kind="ExternalOutput").ap()
        pt_dbg = nc.dram_tensor("pt_dbg", [128, HD * 1024], F32,
                                kind="ExternalOutput").ap()

    mask_eng_env = _os.environ.get("K_MASKDVE")

    with tile.TileContext(nc) as tc:
        with (
            tc.tile_pool(name="persist", bufs=1) as persist,
            tc.tile_pool(name="xin", bufs=4 if dbg else 6) as xinp,
            tc.tile_pool(name="pt", bufs=10) as ptp,
            tc.tile_pool(name="ctx", bufs=4) as ctxp,
            tc.tile_pool(name="ctxT8", bufs=4) as ctxT8p,
            tc.tile_pool(name="small", bufs=8) as smallp,
            tc.tile_pool(name="outs", bufs=4) as outsp,
            tc.tile_pool(name="psS", bufs=2, space="PSUM") as psS,
            tc.tile_pool(name="psAV", bufs=1, space="PSUM") as psAV,
            tc.tile_pool(name="psG", bufs=1, space="PSUM") as psG,
        ):
            # ---- persistent SBUF ----
            # weights: [128, s(hi/lo), ktile, cols]
            wq_s = persist.tile([128, 2, NCH, GW], F8, tag="wq")
            wk_s = persist.tile([128, 2, NCH, GW], F8, tag="wk")
            wv_s = persist.tile([128, 2, NCH, AUGW], F8, tag="wv")
            wo_s = persist.tile([128, 2, 4, D], F8, tag="wo")
            qt_s = persist.tile([128, 4, L], F32R, tag="qt")
            kt_s = persist.tile([128, 4, L], F32R, tag="kt")
            vh_s = persist.tile([128, NKB, 2, 260], BF16, tag="vh")
            bqk_s = persist.tile([128, 8], F32, tag="bqk")
            vb_s = persist.tile([128, AUGW], F32, tag="vb")
            mi_s = persist.tile([128, 256], BF16, tag="mi")

            def emit_w_dmas(which, split=False):
                if which == "bqk":
                    nc.sync.dma_start(bqk_s[:, :], bqk[:, :])
                elif which == "first":
                    nc.sync.dma_start(mi_s[:, :], mi[:, :])
                    vb_bcast = bass.AP(tensor=vb.tensor, offset=vb.offset,
                                       ap=[[0, 128], [1, AUGW]])
                    nc.sync.dma_start(vb_s[:, :], vb_bcast)
                elif which in ("q", "k", "v"):
                    w_s, w_d = {"q": (wq_s, wq), "k": (wk_s, wk),
                                "v": (wv_s, wv)}[which]
                    src = w_d.rearrange("(s c p) q -> p s c q", s=2, p=128)
                    if split:
                        for s in range(2):
                            nc.sync.dma_start(w_s[:, s, :, :], src[:, s, :, :])
                    else:
                        nc.sync.dma_start(w_s[:, :, :, :], src)
                else:
                    nc.sync.dma_start(
                        wo_s[:, :, :, :],
                        wo.rearrange("(s c p) q -> p s c q", s=2, p=128))

            # ---------- pacing counters (ns, at full clocks) ----------
            st = {"pe": 0.0, "act": 0.0}

            def mm(*args, **kw):
                out = args[0]
                st["pe"] += out.free_size() * PE_NS
                nc.tensor.matmul(*args, **kw)

            def mmdr(*args, **kw):
                out = args[0]
                st["pe"] += out.free_size() * 0.5 * PE_NS
                nc.tensor.matmul(*args, perf_mode=DR, **kw)

            # ---------- projection / output-projection units ----------
            def emit_x_dma(n, src, tag, split=False):
                # whole-slice tile [128, s(hi/lo), 8 ktiles, SL], one DMA
                # (two when split: hi first so class-A matmuls start earlier)
                t = xinp.tile([128, 2, NCH, SL], F8, tag="x", name=f"x{tag}{n}")
                s_ap = src.rearrange("(s c p) q -> p s c q", s=2, p=128)[
                    :, :, :, n * SL:(n + 1) * SL]
                if split:
                    for s in range(2):
                        nc.sync.dma_start(t[:, s, :, :], s_ap[:, s, :, :])
                else:
                    nc.sync.dma_start(t[:, :, :, :], s_ap)
                return t

            # 3-slot fp8 classes: (ws, xs) in A=(hi,hi), B=(lo,hi), C=(hi,lo)
            SLOT3 = ((0, 0), (1, 0), (0, 1))

            def emit_qk_unit(n, xt, w_s, dst, b_s, m, ci, psh):
                # ci: slot-class index 0..2 (emission granularity), or None
                cis = range(3) if ci is None else (ci,)
                if cis[0] == 0:
                    psh[m] = psG.tile([128, 512], F32, tag="g", bufs=2,
                                      name=f"qk{n}_{m}")
                ps = psh[m]
                for c3 in cis:
                    ws, xs = SLOT3[c3]
                    for cp in range(4):
                        cg = cp * 2
                        mmdr(ps[:, :],
                             w_s[:, ws, cg:cg + 2, m * 128:(m + 1) * 128],
                             xt[:, xs, cg:cg + 2, :],
                             start=(c3 == 0 and cp == 0),
                             stop=(c3 == 2 and cp == 3))
                if cis[-1] == 2:
                    del psh[m]
                    nc.vector.tensor_scalar_add(
                        dst[:, m, n * SL:(n + 1) * SL], ps[:, :],
                        b_s[:, m:m + 1])

            def emit_v_unit(n, xt, tt, hf, ci, psh):
                cis = range(3) if ci is None else (ci,)
                if cis[0] == 0:
                    psh[(tt, hf)] = psG.tile([128, 512], F32, tag="g", bufs=2,
                                             name=f"v{n}_{tt}_{hf}")
                ps = psh[(tt, hf)]
                for c3 in cis:
                    xs, ws = SLOT3[c3]  # lhsT is x here
                    for cp in range(4):
                        cg = cp * 2
                        mmdr(ps[:, 0:260],
                             xt[:, xs, cg:cg + 2, tt * 128:(tt + 1) * 128],
                             wv_s[:, ws, cg:cg + 2,
                                  hf * 260:(hf + 1) * 260],
                             start=(c3 == 0 and cp == 0),
                             stop=(c3 == 2 and cp == 3))
                if cis[-1] == 2:
                    del psh[(tt, hf)]
                    kb = n * 4 + tt
                    nc.vector.tensor_add(
                        vh_s[:, kb, hf, :],
                        ps[:, 0:260], vb_s[:, hf * 260:(hf + 1) * 260])

            def emit_c_unit(n, tt, n2, ctxT8_n):
                if n >= NS - 1 and (tt + n2) % 2 == 0:
                    # the S psum banks are free once the last exps retire;
                    # alternating pools doubles the tail eviction pipeline
                    ps2 = psS.tile([128, 2, 512], F32, tag="s",
                                   name=f"c{n}_{tt}_{n2}")
                    ps = ps2[:, 0, :]
                else:
                    ps = psG.tile([128, 512], F32, tag="g", bufs=2,
                                  name=f"c{n}_{tt}_{n2}")
                # cp0 slots first (ctx chunks 0-1 are ready before 2-3 at the
                # tail), ctx-lo x cp1 last — gives the final head's hi/lo
                # split maximal runway; psum accumulation order is free
                order = ((0, 0), (1, 0), (2, 0), (0, 1), (2, 1), (1, 1))
                for i, (c3, cp) in enumerate(order):
                    s_ctx, s_wo = SLOT3[c3]
                    mmdr(ps[:, :],
                         ctxT8_n[:, s_ctx, cp * 2:cp * 2 + 2,
                                 tt * 128:(tt + 1) * 128],
                         wo_s[:, s_wo, cp * 2:cp * 2 + 2,
                              n2 * 512:(n2 + 1) * 512],
                         start=(i == 0), stop=(i == 5))
                ot = outsp.tile([128, 512], BF16, tag="ot", name=f"ot{n}_{tt}_{n2}")
                if n >= NS - 1:
                    # Act is done with exps by the time these pop; DVE still
                    # has the last ctxT8 hi/lo work queued
                    nc.scalar.activation(ot[:, :], ps[:, :],
                                         func=mybir.ActivationFunctionType.Copy,
                                         scale=C_EVICT)
                else:
                    nc.vector.tensor_scalar_mul(ot[:, :], ps[:, :], C_EVICT)
                nc.sync.dma_start(
                    outp[(n * 4 + tt) * 128:(n * 4 + tt + 1) * 128,
                         n2 * 512:(n2 + 1) * 512], ot[:, :])

            def c_units(n, ctxT8_n):
                units = []
                for tt in range(4):
                    for n2 in range(2):
                        units.append(lambda n=n, tt=tt, n2=n2: emit_c_unit(
                            n, tt, n2, ctxT8_n))
                return units

            # ---------- prologue: DMAs + Q/K m0 only (act starts ASAP) ----
            # interleaved hi-first DMA order so class-A DR matmuls can start
            # after ~1MB of transfer; the small DMAs queue behind the first
            # consumer-critical MBs (each dma_start costs ~665ns of HWDGE):
            # bqk before the first qt evict, mi/vb before the first mask/V
            # evict (not needed until drain item 10 / V pops)
            wq_src = wq.rearrange("(s c p) q -> p s c q", s=2, p=128)
            wk_src = wk.rearrange("(s c p) q -> p s c q", s=2, p=128)
            xts = {}
            xts["q"] = xinp.tile([128, 2, NCH, SL], F8, tag="x", name="xq0")
            xts["k"] = xinp.tile([128, 2, NCH, SL], F8, tag="x", name="xk0")
            xq_src = xq.rearrange("(s c p) q -> p s c q", s=2, p=128)
            xk_src = xk.rearrange("(s c p) q -> p s c q", s=2, p=128)
            # the very first hi blocks go in ktile-halves: class-A cp0/cp1
            # DRs start after ~0.5MB instead of 1MB
            nc.sync.dma_start(wq_s[:, 0, 0:4, :], wq_src[:, 0, 0:4, :])
            nc.sync.dma_start(xts["q"][:, 0, 0:4, :], xq_src[:, 0, 0:4, 0:SL])
            nc.sync.dma_start(wq_s[:, 0, 4:8, :], wq_src[:, 0, 4:8, :])
            nc.sync.dma_start(xts["q"][:, 0, 4:8, :], xq_src[:, 0, 4:8, 0:SL])
            nc.sync.dma_start(wq_s[:, 1, :, :], wq_src[:, 1, :, :])
            nc.sync.dma_start(xts["q"][:, 1, :, :], xq_src[:, 1, :, 0:SL])
            emit_w_dmas("bqk")
            for s in range(2):
                nc.sync.dma_start(wk_s[:, s, :, :], wk_src[:, s, :, :])
                nc.sync.dma_start(xts["k"][:, s, :, :], xk_src[:, s, :, 0:SL])
            emit_w_dmas("first")
            # full QK(0): the later m-chunks overlap the wk/xk DMA waits
            psh0 = {}
            for m in range(4):
                emit_qk_unit(0, xts["q"], wq_s, qt_s, bqk_s[:, 0:4], m, None, psh0)
            for m in range(4):
                emit_qk_unit(0, xts["k"], wk_s, kt_s, bqk_s[:, 4:8], m, None, psh0)
            emit_w_dmas("v")
            xts["v"] = emit_x_dma(0, xv, "v")

            # ---------- attention + pipeline ----------
            ctxT8_all = {}   # n -> [128, 2, 4, 512] fp8 hi/lo tile

            # Two filler queues paced against the Act exp stream:
            #  - dq: hard-deadline entries (due, thunk), FIFO in due order —
            #    projection units gated just-in-time per m-chunk / slice
            #  - sq: soft entries (deferred C units), popped on margin only
            dq = []
            sq = []
            margin = float(_os.environ.get("K_MARGIN", 2500))

            def pop_fillers(now=None, force_all=False):
                # dq pops strictly at deadline (margin pops would run ahead
                # of the DMA stream and stall the in-order PE); sq (C units)
                # absorbs the margin slack
                while dq and (force_all
                              or (now is not None and dq[0][0] <= now)):
                    dq.pop(0)[1]()
                while sq and (force_all or st["pe"] < st["act"] + margin):
                    sq.pop(0)()

            # prologue PE work ran concurrent with the x/w DMA stream; start
            # the pacing race fresh at the item stream
            st["act"] = st["pe"]

            def gidx(n, ii):
                return n * 1000 + ii

            def queue_qk_m(n1, m, xtd, psh, n_due, ii_due):
                # spread: 2 entries per item starting at (n_due, ii_due)
                ents = []
                for ci in range(3):
                    ents.append(lambda m=m, ci=ci: emit_qk_unit(
                        n1, xtd["q"], wq_s, qt_s, bqk_s[:, 0:4], m, ci, psh))
                for ci in range(3):
                    ents.append(lambda m=m, ci=ci: emit_qk_unit(
                        n1, xtd["k"], wk_s, kt_s, bqk_s[:, 4:8], m, ci, psh))
                for i, e in enumerate(ents):
                    dq.append((gidx(n_due, ii_due + i // 2), e))

            def queue_v(n1, xtd, psh, n_due, ii_due, spread=2):
                # AV consumes the diagonal key blocks (high tt) first and
                # heads 0-3 (hf=0) before 4-7
                ents = [lambda tt=tt, hf=hf: emit_v_unit(
                            n1, xtd["v"], tt, hf, None, psh)
                        for tt in range(4) for hf in range(2)]
                for i, e in enumerate(ents):
                    dq.append((gidx(n_due, ii_due + i // spread), e))

            # V(0) just-in-time within slice 0 (after its DMA lands); wo
            # rides the queue behind the slice-1 x prefetch
            # V(0) pops at items 5-8: after the wv/xv0 DMAs land (no in-order
            # PE stall) and before the first AV drain (at item 10)
            psh0b = {}
            queue_v(0, xts, psh0b, 0, 5)
            dq.append((gidx(0, 9), lambda: emit_w_dmas("rest")))

            for n in range(NS):
                if n + 1 < NS:
                    # eager whole-slice x prefetch + JIT-gated proj units;
                    # V(n+1)+m0(n+1) spread over the tail items of slice n.
                    # xv first: the V units pop before the QK m0 units
                    n1 = n + 1
                    nxts = {}
                    nxts["v"] = emit_x_dma(n1, xv, "v")
                    nxts["q"] = emit_x_dma(n1, xq, "q")
                    nxts["k"] = emit_x_dma(n1, xk, "k")
                    pshn = {}
                    ni = HD * 2 * (n + 1)    # items in slice n
                    npg = 2 * (n1 + 1)       # items per head in slice n1
                    queue_v(n1, nxts, pshn, n, ni - 7)
                    queue_qk_m(n1, 0, nxts, pshn, n, ni - 3)
                    for j in (1, 2, 3):
                        queue_qk_m(n1, j, nxts, pshn,
                                   n1, max(0, 2 * j * npg - 7))

                nkb = 4 * n + 4
                ngrp = nkb // 2
                ctxT8_n = ctxT8p.tile([128, 2, 4, 512], F8, tag="ctxT8",
                                      name=f"ctxT8_{n}")
                ctxT8_all[n] = ctxT8_n
                psT_cur = [None]

                def emit_s_exp(h, g, pts):
                    # Both banks of a group share the group's column base so a
                    # single 2-bank exp covers them (the extra computed scores
                    # land in q-chunks the AV stage never reads).
                    mc, po = h // 2, (h % 2) * 64
                    sps = psS.tile([128, 2, 512], F32, tag="s",
                                   name=f"s{n}_{h}_{g}")
                    pt = ptp.tile([128, 2, 512], BF16, tag="pt",
                                  name=f"pt{n}_{h}_{g}")
                    c0a = max(0, 2 * g * 128 - n * SL)
                    for i in range(2):
                        kb = 2 * g + i
                        # per-kb trim: queries before the key block are dead
                        # (AV skips them); exp still reads from c0a, the
                        # stale-psum cols it covers land in dead pt slots.
                        # f32r needs >=256 moving cols for 1 cyc/row.
                        c0i = min(max(0, kb * 128 - n * SL), SL - 256)
                        mm(sps[:, i, c0i:],
                           kt_s[po:po + 64, mc, kb * 128:(kb + 1) * 128],
                           qt_s[po:po + 64, mc, n * SL + c0i:(n + 1) * SL],
                           start=True, stop=True)
                    if dbg and n == 0 and g == 0:
                        sd = outsp.tile([128, 2, 512], F32, tag="sd", bufs=1,
                                        name=f"sd{n}_{h}_{g}")
                        nc.vector.tensor_copy(sd[:, :, :], sps[:, :, :])
                        nc.sync.dma_start(
                            s_dbg[:, h * 1024:(h + 1) * 1024], sd[:, :, :])
                    st["act"] += (2 * (512 - c0a)) * ACT_NS + 185.0
                    nc.scalar.activation(
                        pt[:, :, c0a:], sps[:, :, c0a:],
                        func=mybir.ActivationFunctionType.Exp,
                        scale=EXP_SCALE)
                    if dbg and n == 0 and g == 0:
                        pd = outsp.tile([128, 2, 512], F32, tag="pd", bufs=1,
                                        name=f"pd{n}_{h}_{g}")
                        nc.vector.tensor_copy(pd[:, :, :], pt[:, :, :])
                        nc.sync.dma_start(
                            pt_dbg[:, h * 1024:(h + 1) * 1024], pd[:, :, :])
                    pts[(h, g)] = pt

                def emit_av(h, g, avp, pts, first_grp, last_grp):
                    # PSUM start=True lazily zero-marks the WHOLE bank, so
                    # only the first emitted write into the bank may use it;
                    # later first-writes per region overwrite via the
                    # pending-zero flags.  Accumulation order over kb is free.
                    pt = pts.pop((h, g))
                    for i in range(2):
                        kb = 2 * g + i
                        if kb >= 4 * n:  # diagonal block: causal mask
                            col0 = max(0, kb * 128 - n * SL)
                            (nc.vector if mask_eng_env
                             else nc.gpsimd).tensor_mul(
                                pt[:, i, col0:col0 + 128],
                                pt[:, i, col0:col0 + 128], mi_s[:, 0:128])
                    started = [not (g == first_grp)]
                    for i in range(2):
                        kb = 2 * g + i
                        for qc in range(4):
                            if kb > 4 * n + qc:
                                continue
                            last = (g == last_grp) and (
                                kb == min(1, 4 * n + qc))
                            mm(avp[:, qc, 0:65],
                               pt[:, i, qc * 128:(qc + 1) * 128],
                               vh_s[:, kb, h // 4, (h % 4) * 65:(h % 4) * 65 + 65],
                               start=not started[0],
                               stop=last,
                               skip_group_check=True)
                            started[0] = True

                def emit_tail(h, avp):
                    # normalize: ctx_t[q, d] = 4 * av[q, d] / (av[q, 64]/8)
                    mc, po = h // 2, (h % 2) * 64
                    rcp = smallp.tile([128, 4, 1], F32, tag="rcp",
                                      name=f"rcp{n}_{h}")
                    nc.vector.reciprocal(rcp[:, :], avp[:, :, 64:65])
                    ctx_t = ctxp.tile([128, 4, DK], BF16, tag="ctx",
                                      name=f"ctx{n}_{h}")
                    if dbg and n == 0:
                        avd = outsp.tile([128, 4, 65], F32, tag="avd", bufs=1,
                                         name=f"avd{n}_{h}")
                        nc.vector.tensor_copy(avd[:, :, :], avp[:, :, :65])
                        nc.sync.dma_start(
                            av_dbg[:, h * 260:(h + 1) * 260], avd[:, :, :])
                    for qc in range(4):
                        nc.vector.tensor_scalar_mul(
                            ctx_t[:, qc, :], avp[:, qc, 0:DK], rcp[:, qc, :])
                    if dbg and n == 0:
                        ctd = outsp.tile([128, 4, DK], F32, tag="ctd", bufs=1,
                                         name=f"ctd{n}_{h}")
                        nc.vector.tensor_copy(ctd[:, :, :], ctx_t[:, :, :])
                        nc.sync.dma_start(
                            ct_dbg[:, h * 256:(h + 1) * 256], ctd[:, :, :])
                    # transpose to dims-major; 2 heads share one psum tile
                    if h % 2 == 0:
                        psT_cur[0] = psG.tile([128, 512], BF16, tag="t",
                                              name=f"t{n}_{mc}")
                    psT = psT_cur[0]
                    for qc in range(4):
                        st["pe"] += 128 * PE_NS
                        nc.tensor.transpose(
                            psT[po:po + 64, qc * 128:(qc + 1) * 128],
                            ctx_t[:, qc, :], mi_s[:, 128:256])
                    if h % 2 == 1:
                        # fp8 hi/lo split of the transposed ctx chunk
                        nc.vector.tensor_copy(ctxT8_n[:, 0, mc, :], psT[:, :])
                        nc.vector.tensor_sub(ctxT8_n[:, 1, mc, :], psT[:, :],
                                             ctxT8_n[:, 0, mc, :])

                # flat (head, group) pipeline: AV lags S/exp by one item so
                # head boundaries don't bunch the Act queue against psS WARs
                gorder = list(range(ngrp - 1, -1, -1))  # diag groups first
                items = [(h, g) for h in range(HD) for g in gorder]
                pts = {}
                avps = {}
                pending = []

                def drain_av():
                    ph, pg = pending.pop(0)
                    emit_av(ph, pg, avps[ph], pts, gorder[0], gorder[-1])
                    if pg == gorder[-1]:
                        emit_tail(ph, avps.pop(ph))

                for ii, (h, g) in enumerate(items):
                    if g == gorder[0]:
                        avps[h] = psAV.tile([128, 4, 128], F32, tag="av",
                                            name=f"av{n}_{h}")
                    emit_s_exp(h, g, pts)
                    pending.append((h, g))
                    # deep lag lets slice-0's V wait out its DMA before the
                    # first AV; the last slice drains shallow so its tails
                    # (and so the C(3) units) finish earlier — there the S
                    # stream is already act-paced, so AV never waits on pt
                    if len(pending) > (9 if n < NS - 1 else 2):
                        drain_av()
                    pop_fillers(now=gidx(n, ii))
                while pending:
                    drain_av()

                # C units for this slice become available now
                sq.extend(c_units(n, ctxT8_n))
                if dbg:
                    for c in range(4):
                        ct = outsp.tile([128, 512], F32, tag="dbg", bufs=1,
                                        name=f"cdb{n}_{c}")
                        h8 = outsp.tile([128, 512], F32, tag="dbg8", bufs=1,
                                        name=f"cdb8{n}_{c}")
                        nc.vector.tensor_copy(h8[:, :], ctxT8_n[:, 0, c, :])
                        nc.vector.tensor_add(ct[:, :], h8[:, :],
                                             ctxT8_n[:, 1, c, :])
                        nc.sync.dma_start(
                            ctx_dbg[:, (n * 4 + c) * 512:(n * 4 + c + 1) * 512],
                            ct[:, :])

            # ---------- drain remaining fillers (incl. all deferred C) ----
            pop_fillers(force_all=True)

            if dbg:
                nc.sync.dma_start(qt_dbg[:, :], qt_s[:, :, :].bitcast(F32))
                nc.sync.dma_start(kt_dbg[:, :], kt_s[:, :, :].bitcast(F32))
                for kb in range(NKB):
                    for hf in range(2):
                        vt = outsp.tile([128, 512], F32, tag="dbg", bufs=1,
                                        name=f"vdb{kb}_{hf}")
                        nc.vector.tensor_copy(vt[:, 0:260], vh_s[:, kb, hf, :])
                        nc.sync.dma_start(
                            vh_dbg[:, kb * 520 + hf * 260:
                                   kb * 520 + (hf + 1) * 260], vt[:, 0:260])

    nc.compile()
    return nc


_NC = None
LAST_RESULTS = None


def _hl(x, f8):
    h = x.astype(f8)
    l = (x - h.astype(np.float32)).astype(f8)
    return np.concatenate([h, l], axis=0)


def kernel(**inputs):
    global _NC, LAST_RESULTS
    import os
    import ml_dtypes
    if _NC is None:
        _NC = _build_nc()

    f8 = ml_dtypes.float8_e4m3
    bf = ml_dtypes.bfloat16
    f = lambda a: np.asarray(a, dtype=np.float32)
    q, k, v = f(inputs["q"]), f(inputs["k"]), f(inputs["v"])
    wq_w, wq_b = f(inputs["wq_w"]), f(inputs["wq_b"])
    wk_w, wk_b = f(inputs["wk_w"]), f(inputs["wk_b"])
    wv_w, wv_b = f(inputs["wv_w"]), f(inputs["wv_b"])
    wo_w, wo_b = f(inputs["wo_w"]), f(inputs["wo_b"])

    msk = np.ascontiguousarray(
        (np.arange(128)[None, :] >= np.arange(128)[:, None])).astype(bf)
    idn = np.eye(128).astype(bf)
    mi = np.ascontiguousarray(np.concatenate([msk, idn], axis=1))

    gmaps = []
    for g in range(2):
        sl = slice(g * GW, (g + 1) * GW)
        wq8 = _hl(np.ascontiguousarray(wq_w[sl].T * SQ), f8)
        wk8 = _hl(np.ascontiguousarray(wk_w[sl].T * SQ), f8)
        wvT = np.zeros((D, AUGW), np.float32)
        vbias = np.zeros((AUGW,), np.float32)
        for h in range(HD):
            wvT[:, h * 65:h * 65 + 64] = wv_w[g * GW + h * 64:
                                              g * GW + (h + 1) * 64].T * SV
            vbias[h * 65:h * 65 + 64] = wv_b[g * GW + h * 64:
                                             g * GW + (h + 1) * 64] * SV
            vbias[h * 65 + 64] = SONE
        wv8 = _hl(wvT, f8)
        wo8 = _hl(np.ascontiguousarray(wo_w[:, sl].T * SO), f8)
        bqT = np.ascontiguousarray((wq_b[sl] * SQ).reshape(4, 128).T)
        bkT = np.ascontiguousarray((wk_b[sl] * SQ).reshape(4, 128).T)
        gmaps.append(dict(wq=wq8, wk=wk8, wv=wv8, wo=wo8,
                          bqk=np.ascontiguousarray(
                              np.concatenate([bqT, bkT], axis=1)),
                          vb=vbias, mi=mi))

    bmaps = []
    for b in range(B):
        bmaps.append(dict(
            xq=_hl(np.ascontiguousarray(q[b].T), f8),
            xk=_hl(np.ascontiguousarray(k[b].T), f8),
            xv=_hl(np.ascontiguousarray(v[b].T), f8)))

    in_maps = [dict(**bmaps[c // 2], **gmaps[c % 2]) for c in range(8)]

    trace = bool(int(os.environ.get("KERNEL_TRACE", "0")))
    res = run_bass_kernel_spmd(_NC, in_maps, list(range(8)), trace=trace)
    LAST_RESULTS = res

    out = np.empty((B, L, D), np.float32)
    for b in range(B):
        out[b] = (np.asarray(res.results[2 * b]["outp"], np.float32)
                  + np.asarray(res.results[2 * b + 1]["outp"], np.float32)
                  + wo_b[None, :])
    return out
